# revision 1
# baseline (speedup 1.0000x reference)
"""Trainium2 Bass kernel for nn_GCNLayer (3-layer GCN + max/mean pooling, T temporal slices).

Self-contained: hardcodes the problem shapes (N=50000, E=800000, B=250, T=8,
CIN=32, COUT=64) and distributes over 8 NeuronCores by graph/dst-node range.

Algorithm per layer (S = sym-normalized adjacency incl. self-loops):
    H_out = relu((S @ H_in) @ W + b)
computed edge-parallel per core:
  - dma_gather of H_in[src] rows (bf16 features for all layers)
  - scatter-add via one-hot matmul: lhsT = O (128 edges x 128 dst slots,
    norm values baked in), rhs = gathered messages, PSUM-accumulated per
    128-node dst block
  - PE transpose -> W matmul (channels on partitions) -> relu+bias on ACT
  - pooling (max + mean over each graph's 200 nodes) via free-dim reduces
  - transpose back, store bf16 H to DRAM, AllGather across the 8 cores
"""

import os
import numpy as np
import ml_dtypes

import concourse.bass as bass
import concourse.mybir as mybir
from concourse import bacc, tile
from concourse.bass_utils import run_bass_kernel_spmd

F32 = mybir.dt.float32
BF16 = mybir.dt.bfloat16
I16 = mybir.dt.int16
P = 128


class Cfg:
    def __init__(self, N=50000, E=800000, B=250, T=8, CIN=32, COUT=64,
                 NCORES=8, GRAPH=200):
        self.N, self.E, self.B, self.T = N, E, B, T
        self.CIN, self.COUT, self.NCORES, self.GRAPH = CIN, COUT, NCORES, GRAPH
        # graphs per core (first cores take the remainder)
        base, rem = divmod(B, NCORES)
        self.gpc = [base + (1 if c < rem else 0) for c in range(NCORES)]
        self.GPC = max(self.gpc)                      # uniform per-core graph slots
        self.NPC = self.GPC * GRAPH                   # padded nodes per core
        assert self.NPC % P == 0
        self.NBLK = self.NPC // P                     # dst blocks per core
        self.NPAD = self.NPC * NCORES                 # padded global node count
        self.HALF = self.NPAD // 2                    # gather index split point
        assert self.HALF <= 32767 + 1
        self.CH1 = CIN * T                            # layer-1 feature row
        self.CH = COUT * T                            # layer-2/3 feature row
        assert self.CH % P == 0
        self.NS = self.CH // P                        # psi partition tiles (t-pairs)
        self.GRP = 4 if self.NBLK >= 4 else self.NBLK  # blocks per processing group
        # graph id offset per core
        self.goff = np.concatenate([[0], np.cumsum(self.gpc)]).astype(np.int64)
        # node range starts in padded space
        self.nstart = [c * self.NPC for c in range(NCORES)]


def _wrap_idx(vals, ncols):
    """int16 index wrap: position i -> [partition i%16, col i//16], replicated to 128."""
    n = len(vals)
    arr = np.zeros((16, ncols), np.int16)
    if n:
        cols = (n + 15) // 16
        buf = np.zeros(cols * 16, np.int64)
        buf[:n] = vals
        arr[:, :cols] = buf.reshape(cols, 16).T
    return np.tile(arr, (8, 1))


def preprocess(cfg, x, edge_index, batch, W1, b1, W2, b2, W3, b3):
    """Build all per-core device inputs. Returns (common_inputs, per_core_inputs, meta)."""
    N, E, T, CIN, COUT = cfg.N, cfg.E, cfg.T, cfg.CIN, cfg.COUT
    src = np.asarray(edge_index[0], np.int64)
    dst = np.asarray(edge_index[1], np.int64)

    # degrees incl self-loops, matching the reference
    deg = np.bincount(dst, minlength=N).astype(np.float32) + 1.0
    dinv = (1.0 / np.sqrt(deg)).astype(np.float32)

    # map real node id -> padded id
    batch = np.asarray(batch, np.int64)
    # nodes are contiguous per graph (batch sorted); node n belongs to graph batch[n]
    # core of graph g:
    g2c = np.zeros(cfg.B, np.int64)
    for c in range(cfg.NCORES):
        g2c[cfg.goff[c]:cfg.goff[c + 1]] = c
    node_graph = batch
    node_core = g2c[node_graph]
    # local index within the core = n - (first node of the core's first graph)
    first_node_of_core = np.array([cfg.goff[c] * cfg.GRAPH for c in range(cfg.NCORES)], np.int64)
    local_n = np.arange(N) - first_node_of_core[node_core]
    CHK = cfg.NPC // 2
    pad_id = np.where(local_n < CHK,
                      node_core * CHK + local_n,
                      cfg.HALF + node_core * CHK + (local_n - CHK))

    srcp = pad_id[src]
    dstc = node_core[dst]
    dstl = local_n[dst]   # local dst within core

    # X permuted to [NPAD, T*CIN] (t-major rows), f32
    Xp = np.zeros((cfg.NPAD, cfg.CH1), np.float32)
    xm = np.moveaxis(np.asarray(x, np.float32), 2, 1).reshape(N, T * CIN)  # [N, t*CIN+c]
    Xp[pad_id] = xm

    # per-core edge bucketing
    KLKH = []
    per_core = []
    for c in range(cfg.NCORES):
        m = dstc == c
        es, ed = srcp[m], dstl[m]
        nv = dinv[src[m]] * dinv[dst[m]]
        # self loops for real local nodes
        ln = np.where(node_core == np.int64(c))[0]
        sl_pad = pad_id[ln]
        sl_local = local_n[ln]
        es = np.concatenate([es, sl_pad])
        ed = np.concatenate([ed, sl_local])
        nv = np.concatenate([nv, dinv[ln] * dinv[ln]])
        blk = ed // P
        half = (es >= cfg.HALF).astype(np.int64)
        order = np.lexsort((es, half, blk))
        per_core.append((es[order], ed[order], nv[order], blk[order], half[order]))
        # chunk requirement per (block, half)
        for b in range(cfg.NBLK):
            mb = blk[order] == b
            hlo = int(((half[order] == 0) & mb).sum())
            hhi = int(((half[order] == 1) & mb).sum())
            KLKH.append((-(-hlo // P), -(-hhi // P)))
    KL = max(max(k[0] for k in KLKH), 1)
    KH = max(max(k[1] for k in KLKH), 1)

    # groups of blocks; per group+half: calls of <=8 chunks
    groups = []
    b0 = 0
    while b0 < cfg.NBLK:
        groups.append(list(range(b0, min(b0 + cfg.GRP, cfg.NBLK))))
        b0 += cfg.GRP

    def call_splits(nch):
        out, pos = [], 0
        while pos < nch:
            k = min(8, nch - pos)
            out.append((pos, k))
            pos += k
        return out

    # call table (shared by all cores/layers): list of (half, group_idx, chunk0_in_group, nchunks)
    calls = []
    for h in (0, 1):
        K = KL if h == 0 else KH
        for gi, blks in enumerate(groups):
            for pos, k in call_splits(len(blks) * K):
                calls.append((h, gi, pos, k))
    NCALLS = len(calls)

    # chunk -> (call, slot) map per (half, group, chunk_in_group)
    chunk_map = {}
    for ci, (h, gi, pos, k) in enumerate(calls):
        for j in range(k):
            chunk_map[(h, gi, pos + j)] = (ci, j)

    # build per-core O (f32 + bf16), idx
    per_core_inputs = []
    for c in range(cfg.NCORES):
        es, ed, nv, blk, half = per_core[c]
        O = np.zeros((NCALLS, P, 8 * P), np.float32)
        idx = np.zeros((P, NCALLS * 64), np.int16)
        for gi, blks in enumerate(groups):
            for h in (0, 1):
                K = KL if h == 0 else KH
                for bi, b in enumerate(blks):
                    m = (blk == b) & (half == h)
                    e_s, e_d, e_n = es[m], ed[m], nv[m]
                    n_e = len(e_s)
                    assert n_e <= K * P
                    for k in range(K):
                        ci, j = chunk_map[(h, gi, bi * K + k)]
                        lo, hi = k * P, min((k + 1) * P, n_e)
                        cnt = max(0, hi - lo)
                        # gather idx values (pad -> row 0)
                        vals = np.zeros(P, np.int64)
                        if cnt:
                            vals[:cnt] = e_s[lo:hi] - (cfg.HALF if h else 0)
                        i0 = j * P
                        # wrap: position i0+p -> [partition (i0+p)%16, col (i0+p)//16]
                        ii = i0 + np.arange(P)
                        idx[ii % 16, ci * 64 + ii // 16] = vals.astype(np.int16)
                        # one-hot
                        if cnt:
                            rows = np.arange(cnt)
                            cols = j * P + (e_d[lo:hi] - b * P)
                            O[ci, rows, cols] = e_n[lo:hi]
        idx[16:] = np.tile(idx[:16], (7, 1))
        per_core_inputs.append({
            "o23": O.astype(ml_dtypes.bfloat16),
            "gidx": idx,
        })

    # pooling piece table per group: (s-invariant) node segments by graph
    # node n (core-local) -> graph gl = n // GRAPH
    pool_pieces = []   # per group: list of (n0, n1, gcol, first_touch)
    seen = set()
    for gi, blks in enumerate(groups):
        n0g = blks[0] * P
        n1g = (blks[-1] + 1) * P
        pieces = []
        n = n0g
        while n < n1g:
            gl = n // cfg.GRAPH
            nend = min((gl + 1) * cfg.GRAPH, n1g)
            ft = gl not in seen
            seen.add(gl)
            pieces.append((n - n0g, nend - n0g, gl, ft))
            n = nend
        pool_pieces.append(pieces)

    # weights: zero-padded [128,128] lhsT variants (row block q*kdim, col block (q%2)*COUT)
    wz = np.zeros((12, P, P), np.float32)
    for li, W in enumerate((W1, W2, W3)):
        W = np.asarray(W, np.float32)
        kdim = W.shape[0]
        nq = P // kdim
        for q in range(nq):
            half = q % 2
            wz[li * 4 + q, q * kdim:(q + 1) * kdim, half * COUT:(half + 1) * COUT] = W

    bias_col = np.zeros((P, 3), np.float32)
    for i, b in enumerate((b1, b2, b3)):
        bias_col[:, i] = np.tile(np.asarray(b, np.float32), P // COUT)

    ident = np.eye(P, dtype=np.float32)
    common = {
        "xp": Xp.astype(ml_dtypes.bfloat16),
        "wz": wz.astype(ml_dtypes.bfloat16),
        "biascol": bias_col,
        "id_f32": ident,
        "id_bf": ident.astype(ml_dtypes.bfloat16),
    }
    meta = dict(KL=KL, KH=KH, calls=calls, chunk_map=chunk_map, groups=groups,
                pool_pieces=pool_pieces, NCALLS=NCALLS,
                w_f32=[np.tile(np.asarray(W, np.float32), (P // np.asarray(W).shape[0], 1))
                       for W in (W1, W2, W3)])
    return common, per_core_inputs, meta


def build(cfg, meta):
    """Construct the Bass/Tile SPMD program."""
    KL, KH, calls, chunk_map = meta["KL"], meta["KH"], meta["calls"], meta["chunk_map"]
    groups, pool_pieces, NCALLS = meta["groups"], meta["pool_pieces"], meta["NCALLS"]
    NS, CH, CH1, T, COUT = cfg.NS, cfg.CH, cfg.CH1, cfg.T, cfg.COUT
    NS1 = max(CH1 // P, 1)
    CIN = cfg.CIN

    nc = bacc.Bacc("TRN2", target_bir_lowering=False, debug=False,
                   num_devices=cfg.NCORES)

    xp = nc.dram_tensor("xp", [cfg.NPAD, CH1], BF16, kind="ExternalInput")
    o23 = nc.dram_tensor("o23", [NCALLS, P, 8 * P], BF16, kind="ExternalInput")
    gidx = nc.dram_tensor("gidx", [P, NCALLS * 64], I16, kind="ExternalInput")
    wz_d = nc.dram_tensor("wz", [12, P, P], BF16, kind="ExternalInput")
    biascol = nc.dram_tensor("biascol", [P, 3], F32, kind="ExternalInput")
    id_f32 = nc.dram_tensor("id_f32", [P, P], F32, kind="ExternalInput")
    id_bf = nc.dram_tensor("id_bf", [P, P], BF16, kind="ExternalInput")
    out = nc.dram_tensor("out", [P, 2 * NS * cfg.GPC], F32, kind="ExternalOutput")

    rg = [list(range(cfg.NCORES))]

    with tile.TileContext(nc) as tc:
        with (
            tc.tile_pool(name="const", bufs=1) as constp,
            tc.tile_pool(name="msg", bufs=3) as msgp,
            tc.tile_pool(name="msgh", bufs=3) as msghp,
            tc.tile_pool(name="otile", bufs=3) as otp,
            tc.tile_pool(name="oth", bufs=3) as othp,
            tc.tile_pool(name="work", bufs=2) as workp,
            tc.tile_pool(name="psig", bufs=2) as psigp,
            tc.tile_pool(name="pool", bufs=1) as poolp,
            tc.tile_pool(name="gps", bufs=4, space="PSUM") as gpsp,
            tc.tile_pool(name="t1ps", bufs=1, space="PSUM") as t1psp,
            tc.tile_pool(name="psips", bufs=2, space="PSUM") as psipsp,
            tc.tile_pool(name="t2ps", bufs=1, space="PSUM") as t2psp,
            tc.tile_pool(name="dram", bufs=1, space="DRAM") as dramp,
        ):
            # ---- constants into SBUF
            idx_sb = constp.tile([P, NCALLS * 64], I16)
            nc.sync.dma_start(out=idx_sb[:], in_=gidx[:])
            wzt = constp.tile([P, 12 * P], BF16, tag="wzt")
            nc.sync.dma_start(
                out=wzt[:].rearrange("p (i m) -> p i m", i=12), in_=wz_d.ap().rearrange("i p m -> p i m"))
            bct = constp.tile([P, 3], F32)
            nc.sync.dma_start(out=bct[:], in_=biascol[:])
            idf = constp.tile([P, P], F32)
            nc.sync.dma_start(out=idf[:], in_=id_f32[:])
            idb = constp.tile([P, P], BF16)
            nc.sync.dma_start(out=idb[:], in_=id_bf[:])

            # ---- pool accumulators
            lmax = poolp.tile([P, NS * cfg.GPC], F32, tag="lmax")
            lsum = poolp.tile([P, NS * cfg.GPC], F32, tag="lsum")
            fmax = poolp.tile([P, NS * cfg.GPC], F32, tag="fmax")
            fsum = poolp.tile([P, NS * cfg.GPC], F32, tag="fsum")
            for _t in (lmax, lsum, fmax, fsum):
                nc.vector.memset(_t[:], 0.0)

            # ---- DRAM intermediates
            h_mine = []
            h_full = []
            for i in range(2):
                hm = dramp.tile([cfg.NPC, CH], BF16, tag=f"hm{i}")
                h_mine.append(hm)
                hf = dramp.tile([cfg.NPAD, CH], BF16, tag=f"hf{i}")
                h_full.append(hf)

            def layer(li):
                """li in {0,1,2}"""
                ch_in = CH1 if li == 0 else CH
                ns_in = NS1 if li == 0 else NS
                kdim = CIN if li == 0 else COUT
                mdt = BF16
                odram = o23
                opool = otp
                if li == 0:
                    src_lo, src_hi = xp[:cfg.HALF, :], xp[cfg.HALF:cfg.NPAD, :]
                else:
                    hsrc = h_full[li - 1]
                    src_lo, src_hi = hsrc[:cfg.HALF, :], hsrc[cfg.HALF:cfg.NPAD, :]

                # calls grouped by group index for emission order
                calls_of_group = {}
                for ci, (h, gi, pos, k) in enumerate(calls):
                    calls_of_group.setdefault(gi, []).append((ci, h, pos, k))

                for gi, blks in enumerate(groups):
                    gtiles = {}
                    for ci, h, pos, k in calls_of_group[gi]:
                        ni = k * P
                        g = (msgp if h == 0 else msghp).tile([P, 8 * ch_in], mdt,
                                                             tag=f"m{h}")
                        nc.gpsimd.dma_gather(
                            out_ap=g[:, :k * ch_in].rearrange("p (c e) -> p c e", e=ch_in),
                            in_ap=(src_lo if h == 0 else src_hi),
                            idxs_ap=idx_sb[:, ci * 64: ci * 64 + max(ni // 16, 1)],
                            num_idxs=ni,
                            num_idxs_reg=ni,
                            elem_size=ch_in,
                        )
                        ot = opool.tile([P, 8 * P], mdt, tag=f"oo{h}")
                        nc.sync.dma_start(out=ot[:, :k * P], in_=odram[ci, :, :k * P])
                        gtiles[ci] = (g, ot)

                    psi_grp = psigp.tile([P, NS * len(blks) * P], F32, tag="psig")
                    for bi, b in enumerate(blks):
                        gps = gpsp.tile([P, ch_in], F32, tag="gps")
                        nmm = KL + KH
                        mm = 0
                        for h in (0, 1):
                            K = KL if h == 0 else KH
                            for k in range(K):
                                ci, j = chunk_map[(h, gi, bi * K + k)]
                                g, ot = gtiles[ci]
                                nc.tensor.matmul(
                                    gps[:],
                                    lhsT=ot[:, j * P:(j + 1) * P],
                                    rhs=g[:, j * ch_in:(j + 1) * ch_in],
                                    start=(mm == 0), stop=(mm == nmm - 1),
                                )
                                mm += 1
                        # ---- epilogue for block b
                        gbf = workp.tile([P, ch_in], F32, tag="gbf")
                        nc.vector.tensor_copy(out=gbf[:], in_=gps[:])
                        t1 = t1psp.tile([P, ns_in * P], F32, tag="t1")
                        for s in range(ns_in):
                            nc.tensor.transpose(
                                t1[:, s * P:(s + 1) * P],
                                gbf[:, s * P:(s + 1) * P], idf[:])
                        gt = workp.tile([P, ns_in * P], BF16, tag="gt")
                        nc.vector.tensor_copy(out=gt[:], in_=t1[:])
                        if "psi" in os.environ.get("GCN_SKIP", ""):
                            continue
                        psi_ps = psipsp.tile([P, NS * P], F32, tag="psip")
                        nq = P // kdim
                        for t_ in range(T):
                            s_out = t_ // 2
                            q = t_ % nq
                            s_in = t_ // nq
                            nc.tensor.matmul(
                                psi_ps[:, s_out * P:(s_out + 1) * P],
                                lhsT=wzt[:, (li * 4 + q) * P:(li * 4 + q + 1) * P],
                                rhs=gt[:, s_in * P:(s_in + 1) * P],
                                start=(t_ % 2 == 0), stop=(t_ % 2 == 1),
                            )
                        # relu + bias (contiguous), then strided copy into group tile
                        psi_sb = workp.tile([P, NS * P], F32, tag="psisb")
                        nc.scalar.activation(
                            psi_sb[:], psi_ps[:],
                            mybir.ActivationFunctionType.Relu,
                            bias=bct[:, li:li + 1],
                        )
                        gwk = len(blks) * P
                        dst_view = psi_grp[:].rearrange(
                            "p (s n) -> p s n", n=gwk)[:, :, bi * P:(bi + 1) * P]
                        nc.vector.tensor_copy(
                            out=dst_view,
                            in_=psi_sb[:].rearrange("p (s n) -> p s n", s=NS))
                        if li < 2 and "t2" not in os.environ.get("GCN_SKIP", ""):
                            t2 = t2psp.tile([P, NS * P], F32, tag="t2")
                            for s in range(NS):
                                nc.tensor.transpose(
                                    t2[:, s * P:(s + 1) * P],
                                    psi_grp[:, s * len(blks) * P + bi * P:
                                            s * len(blks) * P + (bi + 1) * P],
                                    idf[:])
                            hbf = workp.tile([P, CH], BF16, tag="hbf")
                            nc.vector.tensor_copy(out=hbf[:], in_=t2[:])
                            nc.sync.dma_start(
                                out=h_mine[li][b * P:(b + 1) * P, :], in_=hbf[:])

                    # ---- pooling for this group
                    if "pool" in os.environ.get("GCN_SKIP", ""):
                        continue
                    gw = len(blks) * P
                    for s in range(NS):
                        base = s * gw
                        for (n0, n1, gl, ft) in pool_pieces[gi]:
                            seg = psi_grp[:, base + n0: base + n1]
                            if ft:
                                nc.vector.reduce_max(
                                    out=lmax[:, s * cfg.GPC + gl: s * cfg.GPC + gl + 1],
                                    in_=seg, axis=mybir.AxisListType.X)
                                nc.vector.reduce_sum(
                                    out=lsum[:, s * cfg.GPC + gl: s * cfg.GPC + gl + 1],
                                    in_=seg, axis=mybir.AxisListType.X)
                            else:
                                tm = workp.tile([P, 2], F32, tag="ptmp")
                                nc.vector.reduce_max(out=tm[:, 0:1], in_=seg,
                                                     axis=mybir.AxisListType.X)
                                nc.vector.reduce_sum(out=tm[:, 1:2], in_=seg,
                                                     axis=mybir.AxisListType.X)
                                nc.vector.tensor_tensor(
                                    out=lmax[:, s * cfg.GPC + gl: s * cfg.GPC + gl + 1],
                                    in0=lmax[:, s * cfg.GPC + gl: s * cfg.GPC + gl + 1],
                                    in1=tm[:, 0:1], op=mybir.AluOpType.max)
                                nc.vector.tensor_add(
                                    out=lsum[:, s * cfg.GPC + gl: s * cfg.GPC + gl + 1],
                                    in0=lsum[:, s * cfg.GPC + gl: s * cfg.GPC + gl + 1],
                                    in1=tm[:, 1:2])

                    if (li < 2 and int(os.environ.get("GCN_LAYERS", "3")) > li + 1
                            and blks[-1] + 1 >= (cfg.NPC // 2) // P
                            and blks[0] < (cfg.NPC // 2) // P + cfg.GRP
                            and blks[-1] + 1 >= (cfg.NPC // 2) // P):
                        if not hasattr(layer, "_agA"):
                            pass
                        if gi == ((cfg.NPC // 2) // P - 1) // cfg.GRP:
                            chk = cfg.NPC // 2
                            nc.gpsimd.collective_compute(
                                "AllGather", mybir.AluOpType.bypass,
                                replica_groups=rg,
                                ins=[h_mine[li][0:chk, :]],
                                outs=[h_full[li][0:cfg.HALF, :]],
                            )

                # ---- layer end: accumulate pools
                if "pool" in os.environ.get("GCN_SKIP", ""):
                    pass
                elif li == 0:
                    nc.vector.tensor_copy(out=fmax[:], in_=lmax[:])
                    nc.vector.tensor_copy(out=fsum[:], in_=lsum[:])
                else:
                    nc.vector.tensor_add(out=fmax[:], in0=fmax[:], in1=lmax[:])
                    nc.vector.tensor_add(out=fsum[:], in0=fsum[:], in1=lsum[:])

                if li < 2 and int(os.environ.get("GCN_LAYERS", "3")) > li + 1:
                    chk = cfg.NPC // 2
                    nc.gpsimd.collective_compute(
                        "AllGather", mybir.AluOpType.bypass,
                        replica_groups=rg,
                        ins=[h_mine[li][chk:cfg.NPC, :]],
                        outs=[h_full[li][cfg.HALF:cfg.NPAD, :]],
                    )

            _nl = int(os.environ.get("GCN_LAYERS", "3"))
            for _li in range(_nl):
                layer(_li)

            # mean = sum / GRAPH
            nc.vector.tensor_scalar_mul(fsum[:], fsum[:], float(np.float32(1.0 / cfg.GRAPH)))
            osb = workp.tile([P, 2 * NS * cfg.GPC], F32, tag="osb")
            nc.vector.tensor_copy(out=osb[:, :NS * cfg.GPC], in_=fmax[:])
            nc.vector.tensor_copy(out=osb[:, NS * cfg.GPC:], in_=fsum[:])
            nc.sync.dma_start(out=out[:], in_=osb[:])

    nc.compile()
    return nc


def unshard(cfg, results):
    """[NCORES][128, 2*NS*GPC] -> [B, 2*COUT, T] float32."""
    B, T, COUT, NS, GPC = cfg.B, cfg.T, cfg.COUT, cfg.NS, cfg.GPC
    out = np.zeros((B, 2 * COUT, T), np.float32)
    for c in range(cfg.NCORES):
        V = results[c]["out"]
        for gl in range(cfg.gpc[c]):
            g = cfg.goff[c] + gl
            for s in range(NS):
                for half in range(2):
                    t_ = 2 * s + half
                    co = np.arange(COUT)
                    pp = half * COUT + co
                    out[g, co, t_] = V[pp, s * GPC + gl]
                    out[g, COUT + co, t_] = V[pp, NS * GPC + s * GPC + gl]
    return out


_CACHE = {}


def kernel(**inputs):
    cfg = Cfg()
    common, per_core, meta = preprocess(
        cfg, inputs["x"], inputs["edge_index"], inputs["batch"],
        inputs["W1"], inputs["b1"], inputs["W2"], inputs["b2"],
        inputs["W3"], inputs["b3"])
    key = (meta["KL"], meta["KH"])
    if key not in _CACHE:
        _CACHE[key] = build(cfg, meta)
    nc = _CACHE[key]
    in_maps = []
    for c in range(cfg.NCORES):
        m = dict(common)
        m["o23"] = per_core[c]["o23"]
        m["gidx"] = per_core[c]["gidx"]
        in_maps.append(m)
    res = run_bass_kernel_spmd(nc, in_maps, list(range(cfg.NCORES)))
    return unshard(cfg, res.results)



# revision 7
# speedup vs baseline: 1.1947x; 1.1947x over previous
"""Trainium2 Bass kernel for nn_GCNLayer (3-layer GCN + max/mean pooling, T temporal slices).

Self-contained: hardcodes the problem shapes (N=50000, E=800000, B=250, T=8,
CIN=32, COUT=64) and distributes over 8 NeuronCores by graph/dst-node range.

Algorithm per layer (S = sym-normalized adjacency incl. self-loops):
    H_out = relu((S @ H_in) @ W + b)
computed edge-parallel per core:
  - dma_gather of H_in[src] rows (bf16 features for all layers)
  - scatter-add via one-hot matmul: lhsT = O (128 edges x 128 dst slots,
    norm values baked in), rhs = gathered messages, PSUM-accumulated per
    128-node dst block
  - PE transpose -> W matmul (channels on partitions) -> relu+bias on ACT
  - pooling (max + mean over each graph's 200 nodes) via free-dim reduces
  - transpose back, store bf16 H to DRAM, AllGather across the 8 cores
"""

import os
import numpy as np
import ml_dtypes

import concourse.bass as bass
import concourse.mybir as mybir
from concourse import bacc, tile
from concourse.bass_utils import run_bass_kernel_spmd

F32 = mybir.dt.float32
BF16 = mybir.dt.bfloat16
I16 = mybir.dt.int16
P = 128


class Cfg:
    def __init__(self, N=50000, E=800000, B=250, T=8, CIN=32, COUT=64,
                 NCORES=8, GRAPH=200):
        self.N, self.E, self.B, self.T = N, E, B, T
        self.CIN, self.COUT, self.NCORES, self.GRAPH = CIN, COUT, NCORES, GRAPH
        # graphs per core (first cores take the remainder)
        base, rem = divmod(B, NCORES)
        self.gpc = [base + (1 if c < rem else 0) for c in range(NCORES)]
        self.GPC = max(self.gpc)                      # uniform per-core graph slots
        self.NPC = self.GPC * GRAPH                   # padded nodes per core
        assert self.NPC % P == 0
        self.NBLK = self.NPC // P                     # dst blocks per core
        self.NPAD = self.NPC * NCORES                 # padded global node count
        self.HALF = self.NPAD // 2                    # gather index split point
        assert self.HALF <= 32767 + 1
        self.CH1 = CIN * T                            # layer-1 feature row
        self.CH = COUT * T                            # layer-2/3 feature row
        assert self.CH % P == 0
        self.NS = self.CH // P                        # psi partition tiles (t-pairs)
        self.GRP = 4 if self.NBLK >= 4 else self.NBLK  # blocks per processing group
        # graph id offset per core
        self.goff = np.concatenate([[0], np.cumsum(self.gpc)]).astype(np.int64)
        # node range starts in padded space
        self.nstart = [c * self.NPC for c in range(NCORES)]


def _wrap_idx(vals, ncols):
    """int16 index wrap: position i -> [partition i%16, col i//16], replicated to 128."""
    n = len(vals)
    arr = np.zeros((16, ncols), np.int16)
    if n:
        cols = (n + 15) // 16
        buf = np.zeros(cols * 16, np.int64)
        buf[:n] = vals
        arr[:, :cols] = buf.reshape(cols, 16).T
    return np.tile(arr, (8, 1))


def preprocess(cfg, x, edge_index, batch, W1, b1, W2, b2, W3, b3):
    """Build all per-core device inputs. Returns (common_inputs, per_core_inputs, meta)."""
    N, E, T, CIN, COUT = cfg.N, cfg.E, cfg.T, cfg.CIN, cfg.COUT
    src = np.asarray(edge_index[0], np.int64)
    dst = np.asarray(edge_index[1], np.int64)

    # degrees incl self-loops, matching the reference
    deg = np.bincount(dst, minlength=N).astype(np.float32) + 1.0
    dinv = (1.0 / np.sqrt(deg)).astype(np.float32)

    # map real node id -> padded id
    batch = np.asarray(batch, np.int64)
    # nodes are contiguous per graph (batch sorted); node n belongs to graph batch[n]
    # core of graph g:
    g2c = np.zeros(cfg.B, np.int64)
    for c in range(cfg.NCORES):
        g2c[cfg.goff[c]:cfg.goff[c + 1]] = c
    node_graph = batch
    node_core = g2c[node_graph]
    # local index within the core = n - (first node of the core's first graph)
    first_node_of_core = np.array([cfg.goff[c] * cfg.GRAPH for c in range(cfg.NCORES)], np.int64)
    local_n = np.arange(N) - first_node_of_core[node_core]
    CHK = cfg.NPC // 2
    pad_id = np.where(local_n < CHK,
                      node_core * CHK + local_n,
                      cfg.HALF + node_core * CHK + (local_n - CHK))

    srcp = pad_id[src]
    dstc = node_core[dst]
    dstl = local_n[dst]   # local dst within core

    # X permuted to [NPAD, T*CIN] (t-major rows), f32
    Xp = np.zeros((cfg.NPAD, cfg.CH1), np.float32)
    xm = np.moveaxis(np.asarray(x, np.float32), 2, 1).reshape(N, T * CIN)  # [N, t*CIN+c]
    Xp[pad_id] = xm

    # per-core edge bucketing
    KLKH = []
    per_core = []
    for c in range(cfg.NCORES):
        m = dstc == c
        es, ed = srcp[m], dstl[m]
        nv = dinv[src[m]] * dinv[dst[m]]
        # self loops for real local nodes
        ln = np.where(node_core == np.int64(c))[0]
        sl_pad = pad_id[ln]
        sl_local = local_n[ln]
        es = np.concatenate([es, sl_pad])
        ed = np.concatenate([ed, sl_local])
        nv = np.concatenate([nv, dinv[ln] * dinv[ln]])
        blk = ed // P
        half = (es >= cfg.HALF).astype(np.int64)
        order = np.lexsort((es, half, blk))
        per_core.append((es[order], ed[order], nv[order], blk[order], half[order]))
        # chunk requirement per (block, half)
        for b in range(cfg.NBLK):
            mb = blk[order] == b
            hlo = int(((half[order] == 0) & mb).sum())
            hhi = int(((half[order] == 1) & mb).sum())
            KLKH.append((-(-hlo // P), -(-hhi // P)))
    KL = max(max(k[0] for k in KLKH), 1)
    KH = max(max(k[1] for k in KLKH), 1)

    # groups of blocks; per group+half: calls of <=8 chunks
    groups = []
    b0 = 0
    while b0 < cfg.NBLK:
        groups.append(list(range(b0, min(b0 + cfg.GRP, cfg.NBLK))))
        b0 += cfg.GRP

    def call_splits(nch):
        out, pos = [], 0
        while pos < nch:
            k = min(8, nch - pos)
            out.append((pos, k))
            pos += k
        return out

    # call table (shared by all cores/layers): list of (half, group_idx, chunk0_in_group, nchunks)
    calls = []
    for h in (0, 1):
        K = KL if h == 0 else KH
        for gi, blks in enumerate(groups):
            for pos, k in call_splits(len(blks) * K):
                calls.append((h, gi, pos, k))
    NCALLS = len(calls)

    # chunk -> (call, slot) map per (half, group, chunk_in_group)
    chunk_map = {}
    for ci, (h, gi, pos, k) in enumerate(calls):
        for j in range(k):
            chunk_map[(h, gi, pos + j)] = (ci, j)

    # build per-core O (f32 + bf16), idx
    per_core_inputs = []
    for c in range(cfg.NCORES):
        es, ed, nv, blk, half = per_core[c]
        O = np.zeros((NCALLS, P, 8 * P), np.float32)
        idx = np.zeros((P, NCALLS * 64), np.int16)
        for gi, blks in enumerate(groups):
            for h in (0, 1):
                K = KL if h == 0 else KH
                for bi, b in enumerate(blks):
                    m = (blk == b) & (half == h)
                    e_s, e_d, e_n = es[m], ed[m], nv[m]
                    n_e = len(e_s)
                    assert n_e <= K * P
                    for k in range(K):
                        ci, j = chunk_map[(h, gi, bi * K + k)]
                        lo, hi = k * P, min((k + 1) * P, n_e)
                        cnt = max(0, hi - lo)
                        # gather idx values (pad -> row 0)
                        vals = np.zeros(P, np.int64)
                        if cnt:
                            vals[:cnt] = e_s[lo:hi] - (cfg.HALF if h else 0)
                        i0 = j * P
                        # wrap: position i0+p -> [partition (i0+p)%16, col (i0+p)//16]
                        ii = i0 + np.arange(P)
                        idx[ii % 16, ci * 64 + ii // 16] = vals.astype(np.int16)
                        # one-hot
                        if cnt:
                            rows = np.arange(cnt)
                            cols = j * P + (e_d[lo:hi] - b * P)
                            O[ci, rows, cols] = e_n[lo:hi]
        idx[16:] = np.tile(idx[:16], (7, 1))
        per_core_inputs.append({
            "o23": O.astype(ml_dtypes.bfloat16),
            "gidx": idx,
        })

    # pooling piece table per group: (s-invariant) node segments by graph
    # node n (core-local) -> graph gl = n // GRAPH
    pool_pieces = []   # per group: list of (n0, n1, gcol, first_touch)
    seen = set()
    for gi, blks in enumerate(groups):
        n0g = blks[0] * P
        n1g = (blks[-1] + 1) * P
        pieces = []
        n = n0g
        while n < n1g:
            gl = n // cfg.GRAPH
            nend = min((gl + 1) * cfg.GRAPH, n1g)
            ft = gl not in seen
            seen.add(gl)
            pieces.append((n - n0g, nend - n0g, gl, ft))
            n = nend
        pool_pieces.append(pieces)

    # weights: zero-padded [128,128] lhsT variants (row block q*kdim, col block (q%2)*COUT)
    wz = np.zeros((12, P, P), np.float32)
    for li, W in enumerate((W1, W2, W3)):
        W = np.asarray(W, np.float32)
        kdim = W.shape[0]
        nq = P // kdim
        for q in range(nq):
            half = q % 2
            wz[li * 4 + q, q * kdim:(q + 1) * kdim, half * COUT:(half + 1) * COUT] = W

    bias_col = np.zeros((P, 3), np.float32)
    for i, b in enumerate((b1, b2, b3)):
        bias_col[:, i] = np.tile(np.asarray(b, np.float32), P // COUT)

    ident = np.eye(P, dtype=np.float32)
    common = {
        "xp": Xp.astype(ml_dtypes.bfloat16),
        "wz": wz.astype(ml_dtypes.bfloat16),
        "biascol": bias_col,
        "id_f32": ident,
        "id_bf": ident.astype(ml_dtypes.bfloat16),
    }
    meta = dict(KL=KL, KH=KH, calls=calls, chunk_map=chunk_map, groups=groups,
                pool_pieces=pool_pieces, NCALLS=NCALLS,
                w_f32=[np.tile(np.asarray(W, np.float32), (P // np.asarray(W).shape[0], 1))
                       for W in (W1, W2, W3)])
    return common, per_core_inputs, meta


def build(cfg, meta):
    """Construct the Bass/Tile SPMD program."""
    KL, KH, calls, chunk_map = meta["KL"], meta["KH"], meta["calls"], meta["chunk_map"]
    groups, pool_pieces, NCALLS = meta["groups"], meta["pool_pieces"], meta["NCALLS"]
    NS, CH, CH1, T, COUT = cfg.NS, cfg.CH, cfg.CH1, cfg.T, cfg.COUT
    NS1 = max(CH1 // P, 1)
    CIN = cfg.CIN

    NQ = 4
    nc = bacc.Bacc("TRN2", target_bir_lowering=False, debug=False,
                   num_devices=cfg.NCORES, num_swdge_queues=NQ)

    xp = nc.dram_tensor("xp", [cfg.NPAD, CH1], BF16, kind="ExternalInput")
    o23 = nc.dram_tensor("o23", [NCALLS, P, 8 * P], BF16, kind="ExternalInput")
    gidx = nc.dram_tensor("gidx", [P, NCALLS * 64], I16, kind="ExternalInput")
    wz_d = nc.dram_tensor("wz", [12, P, P], BF16, kind="ExternalInput")
    biascol = nc.dram_tensor("biascol", [P, 3], F32, kind="ExternalInput")
    id_f32 = nc.dram_tensor("id_f32", [P, P], F32, kind="ExternalInput")
    id_bf = nc.dram_tensor("id_bf", [P, P], BF16, kind="ExternalInput")
    out = nc.dram_tensor("out", [P, 2 * NS * cfg.GPC], F32, kind="ExternalOutput")

    rg = [list(range(cfg.NCORES))]

    qstate = {"i": 0}

    with tile.TileContext(nc) as tc:
        with (
            tc.tile_pool(name="const", bufs=1) as constp,
            tc.tile_pool(name="msg", bufs=4) as msgp,
            tc.tile_pool(name="msgh", bufs=4) as msghp,
            tc.tile_pool(name="otile", bufs=3) as otp,
            tc.tile_pool(name="oth", bufs=3) as othp,
            tc.tile_pool(name="work", bufs=2) as workp,
            tc.tile_pool(name="psig", bufs=2) as psigp,
            tc.tile_pool(name="pool", bufs=1) as poolp,
            tc.tile_pool(name="gps", bufs=4, space="PSUM") as gpsp,
            tc.tile_pool(name="t1ps", bufs=1, space="PSUM") as t1psp,
            tc.tile_pool(name="psips", bufs=2, space="PSUM") as psipsp,
            tc.tile_pool(name="t2ps", bufs=1, space="PSUM") as t2psp,
            tc.tile_pool(name="dram", bufs=1, space="DRAM") as dramp,
        ):
            # ---- constants into SBUF
            idx_sb = constp.tile([P, NCALLS * 64], I16)
            nc.sync.dma_start(out=idx_sb[:], in_=gidx[:])
            wzt = constp.tile([P, 12 * P], BF16, tag="wzt")
            nc.sync.dma_start(
                out=wzt[:].rearrange("p (i m) -> p i m", i=12), in_=wz_d.ap().rearrange("i p m -> p i m"))
            bct = constp.tile([P, 3], F32)
            nc.sync.dma_start(out=bct[:], in_=biascol[:])
            idf = constp.tile([P, P], F32)
            nc.sync.dma_start(out=idf[:], in_=id_f32[:])
            idb = constp.tile([P, P], BF16)
            nc.sync.dma_start(out=idb[:], in_=id_bf[:])

            # ---- pool accumulators
            lmax = poolp.tile([P, NS * cfg.GPC], F32, tag="lmax")
            lsum = poolp.tile([P, NS * cfg.GPC], F32, tag="lsum")
            fmax = poolp.tile([P, NS * cfg.GPC], F32, tag="fmax")
            fsum = poolp.tile([P, NS * cfg.GPC], F32, tag="fsum")
            for _t in (lmax, lsum, fmax, fsum):
                nc.vector.memset(_t[:], 0.0)

            # ---- DRAM intermediates
            h_mine = []
            h_full = []
            for i in range(2):
                hm = dramp.tile([cfg.NPC, CH], BF16, tag=f"hm{i}")
                h_mine.append(hm)
                hf = dramp.tile([cfg.NPAD, CH], BF16, tag=f"hf{i}")
                h_full.append(hf)

            def layer(li):
                """li in {0,1,2}"""
                ch_in = CH1 if li == 0 else CH
                ns_in = NS1 if li == 0 else NS
                kdim = CIN if li == 0 else COUT
                mdt = BF16
                odram = o23
                opool = otp
                if li == 0:
                    src_lo, src_hi = xp[:cfg.HALF, :], xp[cfg.HALF:cfg.NPAD, :]
                else:
                    hsrc = h_full[li - 1]
                    src_lo, src_hi = hsrc[:cfg.HALF, :], hsrc[cfg.HALF:cfg.NPAD, :]

                # calls grouped by group index for emission order
                calls_of_group = {}
                for ci, (h, gi, pos, k) in enumerate(calls):
                    calls_of_group.setdefault(gi, []).append((ci, h, pos, k))

                for gi, blks in enumerate(groups):
                    gtiles = {}
                    for ci, h, pos, k in calls_of_group[gi]:
                        ni = k * P
                        g = (msgp if h == 0 else msghp).tile([P, 8 * ch_in], mdt,
                                                             tag=f"m{h}")
                        q = qstate["i"] % NQ
                        qstate["i"] += 1
                        nc.gpsimd.dma_gather(
                            out_ap=g[:, :k * ch_in].rearrange("p (c e) -> p c e", e=ch_in),
                            in_ap=(src_lo if h == 0 else src_hi),
                            idxs_ap=idx_sb[:, ci * 64: ci * 64 + max(ni // 16, 1)],
                            num_idxs=ni,
                            num_idxs_reg=ni,
                            elem_size=ch_in,
                            queue_num=q,
                        )
                        ot = opool.tile([P, 8 * P], mdt, tag=f"oo{h}")
                        nc.sync.dma_start(out=ot[:, :k * P], in_=odram[ci, :, :k * P])
                        gtiles[ci] = (g, ot)

                    psi_grp = psigp.tile([P, NS * len(blks) * P], F32, tag="psig")
                    for bi, b in enumerate(blks):
                        gps = gpsp.tile([P, ch_in], F32, tag="gps")
                        nmm = KL + KH
                        mm = 0
                        for h in (0, 1):
                            K = KL if h == 0 else KH
                            for k in range(K):
                                ci, j = chunk_map[(h, gi, bi * K + k)]
                                g, ot = gtiles[ci]
                                nc.tensor.matmul(
                                    gps[:],
                                    lhsT=ot[:, j * P:(j + 1) * P],
                                    rhs=g[:, j * ch_in:(j + 1) * ch_in],
                                    start=(mm == 0), stop=(mm == nmm - 1),
                                )
                                mm += 1
                        # ---- epilogue for block b
                        gbf = workp.tile([P, ch_in], F32, tag="gbf")
                        nc.vector.tensor_copy(out=gbf[:], in_=gps[:])
                        t1 = t1psp.tile([P, ns_in * P], F32, tag="t1")
                        for s in range(ns_in):
                            nc.tensor.transpose(
                                t1[:, s * P:(s + 1) * P],
                                gbf[:, s * P:(s + 1) * P], idf[:])
                        gt = workp.tile([P, ns_in * P], BF16, tag="gt")
                        nc.vector.tensor_copy(out=gt[:], in_=t1[:])
                        if "psi" in os.environ.get("GCN_SKIP", ""):
                            continue
                        psi_ps = psipsp.tile([P, NS * P], F32, tag="psip")
                        nq = P // kdim
                        for t_ in range(T):
                            s_out = t_ // 2
                            q = t_ % nq
                            s_in = t_ // nq
                            nc.tensor.matmul(
                                psi_ps[:, s_out * P:(s_out + 1) * P],
                                lhsT=wzt[:, (li * 4 + q) * P:(li * 4 + q + 1) * P],
                                rhs=gt[:, s_in * P:(s_in + 1) * P],
                                start=(t_ % 2 == 0), stop=(t_ % 2 == 1),
                            )
                        # relu + bias (contiguous), then strided copy into group tile
                        psi_sb = workp.tile([P, NS * P], F32, tag="psisb")
                        nc.scalar.activation(
                            psi_sb[:], psi_ps[:],
                            mybir.ActivationFunctionType.Relu,
                            bias=bct[:, li:li + 1],
                        )
                        gwk = len(blks) * P
                        dst_view = psi_grp[:].rearrange(
                            "p (s n) -> p s n", n=gwk)[:, :, bi * P:(bi + 1) * P]
                        nc.vector.tensor_copy(
                            out=dst_view,
                            in_=psi_sb[:].rearrange("p (s n) -> p s n", s=NS))
                        if li < 2 and "t2" not in os.environ.get("GCN_SKIP", ""):
                            t2 = t2psp.tile([P, NS * P], F32, tag="t2")
                            for s in range(NS):
                                nc.tensor.transpose(
                                    t2[:, s * P:(s + 1) * P],
                                    psi_grp[:, s * len(blks) * P + bi * P:
                                            s * len(blks) * P + (bi + 1) * P],
                                    idf[:])
                            hbf = workp.tile([P, CH], BF16, tag="hbf")
                            nc.vector.tensor_copy(out=hbf[:], in_=t2[:])
                            nc.sync.dma_start(
                                out=h_mine[li][b * P:(b + 1) * P, :], in_=hbf[:])

                    # ---- pooling for this group
                    if "pool" in os.environ.get("GCN_SKIP", ""):
                        continue
                    gw = len(blks) * P
                    for s in range(NS):
                        base = s * gw
                        for (n0, n1, gl, ft) in pool_pieces[gi]:
                            seg = psi_grp[:, base + n0: base + n1]
                            if ft:
                                nc.vector.reduce_max(
                                    out=lmax[:, s * cfg.GPC + gl: s * cfg.GPC + gl + 1],
                                    in_=seg, axis=mybir.AxisListType.X)
                                nc.vector.reduce_sum(
                                    out=lsum[:, s * cfg.GPC + gl: s * cfg.GPC + gl + 1],
                                    in_=seg, axis=mybir.AxisListType.X)
                            else:
                                tm = workp.tile([P, 2], F32, tag="ptmp")
                                nc.vector.reduce_max(out=tm[:, 0:1], in_=seg,
                                                     axis=mybir.AxisListType.X)
                                nc.vector.reduce_sum(out=tm[:, 1:2], in_=seg,
                                                     axis=mybir.AxisListType.X)
                                nc.vector.tensor_tensor(
                                    out=lmax[:, s * cfg.GPC + gl: s * cfg.GPC + gl + 1],
                                    in0=lmax[:, s * cfg.GPC + gl: s * cfg.GPC + gl + 1],
                                    in1=tm[:, 0:1], op=mybir.AluOpType.max)
                                nc.vector.tensor_add(
                                    out=lsum[:, s * cfg.GPC + gl: s * cfg.GPC + gl + 1],
                                    in0=lsum[:, s * cfg.GPC + gl: s * cfg.GPC + gl + 1],
                                    in1=tm[:, 1:2])

                    if (li < 2 and int(os.environ.get("GCN_LAYERS", "3")) > li + 1
                            and blks[-1] + 1 >= (cfg.NPC // 2) // P
                            and blks[0] < (cfg.NPC // 2) // P + cfg.GRP
                            and blks[-1] + 1 >= (cfg.NPC // 2) // P):
                        if not hasattr(layer, "_agA"):
                            pass
                        if gi == ((cfg.NPC // 2) // P - 1) // cfg.GRP:
                            chk = cfg.NPC // 2
                            nc.gpsimd.collective_compute(
                                "AllGather", mybir.AluOpType.bypass,
                                replica_groups=rg,
                                ins=[h_mine[li][0:chk, :]],
                                outs=[h_full[li][0:cfg.HALF, :]],
                            )

                # ---- layer end: accumulate pools
                if "pool" in os.environ.get("GCN_SKIP", ""):
                    pass
                elif li == 0:
                    nc.vector.tensor_copy(out=fmax[:], in_=lmax[:])
                    nc.vector.tensor_copy(out=fsum[:], in_=lsum[:])
                else:
                    nc.vector.tensor_add(out=fmax[:], in0=fmax[:], in1=lmax[:])
                    nc.vector.tensor_add(out=fsum[:], in0=fsum[:], in1=lsum[:])

                if li < 2 and int(os.environ.get("GCN_LAYERS", "3")) > li + 1:
                    chk = cfg.NPC // 2
                    nc.gpsimd.collective_compute(
                        "AllGather", mybir.AluOpType.bypass,
                        replica_groups=rg,
                        ins=[h_mine[li][chk:cfg.NPC, :]],
                        outs=[h_full[li][cfg.HALF:cfg.NPAD, :]],
                    )

            _nl = int(os.environ.get("GCN_LAYERS", "3"))
            for _li in range(_nl):
                layer(_li)

            # mean = sum / GRAPH
            nc.vector.tensor_scalar_mul(fsum[:], fsum[:], float(np.float32(1.0 / cfg.GRAPH)))
            osb = workp.tile([P, 2 * NS * cfg.GPC], F32, tag="osb")
            nc.vector.tensor_copy(out=osb[:, :NS * cfg.GPC], in_=fmax[:])
            nc.vector.tensor_copy(out=osb[:, NS * cfg.GPC:], in_=fsum[:])
            nc.sync.dma_start(out=out[:], in_=osb[:])

    nc.compile()
    return nc


def unshard(cfg, results):
    """[NCORES][128, 2*NS*GPC] -> [B, 2*COUT, T] float32."""
    B, T, COUT, NS, GPC = cfg.B, cfg.T, cfg.COUT, cfg.NS, cfg.GPC
    out = np.zeros((B, 2 * COUT, T), np.float32)
    for c in range(cfg.NCORES):
        V = results[c]["out"]
        for gl in range(cfg.gpc[c]):
            g = cfg.goff[c] + gl
            for s in range(NS):
                for half in range(2):
                    t_ = 2 * s + half
                    co = np.arange(COUT)
                    pp = half * COUT + co
                    out[g, co, t_] = V[pp, s * GPC + gl]
                    out[g, COUT + co, t_] = V[pp, NS * GPC + s * GPC + gl]
    return out


_CACHE = {}


def kernel(**inputs):
    cfg = Cfg()
    common, per_core, meta = preprocess(
        cfg, inputs["x"], inputs["edge_index"], inputs["batch"],
        inputs["W1"], inputs["b1"], inputs["W2"], inputs["b2"],
        inputs["W3"], inputs["b3"])
    key = (meta["KL"], meta["KH"])
    if key not in _CACHE:
        _CACHE[key] = build(cfg, meta)
    nc = _CACHE[key]
    in_maps = []
    for c in range(cfg.NCORES):
        m = dict(common)
        m["o23"] = per_core[c]["o23"]
        m["gidx"] = per_core[c]["gidx"]
        in_maps.append(m)
    res = run_bass_kernel_spmd(nc, in_maps, list(range(cfg.NCORES)))
    return unshard(cfg, res.results)



# revision 10
# speedup vs baseline: 1.6334x; 1.3672x over previous
"""Trainium2 Bass kernel for nn_GCNLayer (3-layer GCN + max/mean pooling, T temporal slices).

Self-contained: hardcodes the problem shapes (N=50000, E=800000, B=250, T=8,
CIN=32, COUT=64) and distributes over 8 NeuronCores by graph/dst-node range.

Algorithm per layer, with S = D^-1/2 (A+I) D^-1/2 and H' = D^-1/2 H:
    H_out = relu(dinv_dst * (sum_edges H'[src]) @ W + b),  H'_out = dinv * H_out
computed edge-parallel per core:
  - dma_gather of H'[src] rows (bf16, pre-scaled by dinv) on 4 SWDGE queues
  - scatter-add via BINARY one-hot matmul: lhsT one-hots are generated
    on-chip (DVE is_equal against an iota table), PSUM-accumulated per
    128-node dst block; per-block dinv[dst] scale folded into the PSUM copy
  - PE transpose -> W matmul (channels on partitions) -> relu+bias on ACT
  - pooling (max + mean over each graph's 200 nodes) via free-dim reduces
  - transpose back, scale by dinv, store bf16 H' to DRAM, AllGather halves
"""

import os
import numpy as np
import ml_dtypes

import concourse.bass as bass
import concourse.mybir as mybir
from concourse import bacc, tile
from concourse.bass_utils import run_bass_kernel_spmd

F32 = mybir.dt.float32
BF16 = mybir.dt.bfloat16
I16 = mybir.dt.int16
P = 128
MAXREC = 16          # max one-hot records per gather call


class Cfg:
    def __init__(self, N=50000, E=800000, B=250, T=8, CIN=32, COUT=64,
                 NCORES=8, GRAPH=200):
        self.N, self.E, self.B, self.T = N, E, B, T
        self.CIN, self.COUT, self.NCORES, self.GRAPH = CIN, COUT, NCORES, GRAPH
        base, rem = divmod(B, NCORES)
        self.gpc = [base + (1 if c < rem else 0) for c in range(NCORES)]
        self.GPC = max(self.gpc)                      # uniform per-core graph slots
        self.NPC = self.GPC * GRAPH                   # padded nodes per core
        assert self.NPC % P == 0
        self.NBLK = self.NPC // P                     # dst blocks per core
        self.NPAD = self.NPC * NCORES                 # padded global node count
        self.HALF = self.NPAD // 2                    # gather index split point
        assert self.HALF <= 32767 + 1
        self.CH1 = CIN * T                            # layer-1 feature row
        self.CH = COUT * T                            # layer-2/3 feature row
        assert self.CH % P == 0
        self.NS = self.CH // P                        # psi partition tiles (t-pairs)
        self.GRP = 4 if self.NBLK >= 4 else self.NBLK  # blocks per processing group
        self.goff = np.concatenate([[0], np.cumsum(self.gpc)]).astype(np.int64)
        self.nstart = [c * self.NPC for c in range(NCORES)]


def preprocess(cfg, x, edge_index, batch, W1, b1, W2, b2, W3, b3):
    """Build all per-core device inputs. Returns (common_inputs, per_core_inputs, meta)."""
    N, E, T, CIN, COUT = cfg.N, cfg.E, cfg.T, cfg.CIN, cfg.COUT
    NC = cfg.NCORES
    src = np.asarray(edge_index[0], np.int64)
    dst = np.asarray(edge_index[1], np.int64)

    deg = np.bincount(dst, minlength=N).astype(np.float32) + 1.0
    dinv = (1.0 / np.sqrt(deg)).astype(np.float32)

    batch = np.asarray(batch, np.int64)
    g2c = np.zeros(cfg.B, np.int64)
    for c in range(NC):
        g2c[cfg.goff[c]:cfg.goff[c + 1]] = c
    node_core = g2c[batch]
    first_node_of_core = np.array([cfg.goff[c] * cfg.GRAPH for c in range(NC)], np.int64)
    local_n = np.arange(N) - first_node_of_core[node_core]
    CHK = cfg.NPC // 2
    pad_id = np.where(local_n < CHK,
                      node_core * CHK + local_n,
                      cfg.HALF + node_core * CHK + (local_n - CHK))

    srcp = pad_id[src]
    dstc = node_core[dst]
    dstl = local_n[dst]

    # X permuted to [NPAD, T*CIN] rows, PRE-SCALED by dinv (H' convention)
    Xp = np.zeros((cfg.NPAD, cfg.CH1), np.float32)
    xm = np.moveaxis(np.asarray(x, np.float32), 2, 1).reshape(N, T * CIN)
    Xp[pad_id] = xm * dinv[:, None]

    # dinv per core block layout [128, NBLK]: dinv of node b*128+p (1.0 for pads)
    dinvb = np.ones((NC, P, cfg.NBLK), np.float32)
    for c in range(NC):
        ln = np.where(node_core == c)[0]
        dv = np.ones(cfg.NPC, np.float32)
        dv[local_n[ln]] = dinv[ln]
        dinvb[c] = dv.reshape(cfg.NBLK, P).T

    # groups of blocks
    groups = []
    b0 = 0
    while b0 < cfg.NBLK:
        groups.append(list(range(b0, min(b0 + cfg.GRP, cfg.NBLK))))
        b0 += cfg.GRP

    # ---- per-(core, group, half) sorted edge lists (src pad-id, dst local)
    # Self-loops included as ordinary binary edges.
    eg = {}
    for c in range(NC):
        m = dstc == c
        es, ed = srcp[m], dstl[m]
        ln = np.where(node_core == c)[0]
        es = np.concatenate([es, pad_id[ln]])
        ed = np.concatenate([ed, local_n[ln]])
        blk = ed // P
        half = (es >= cfg.HALF).astype(np.int64)
        gidx_of_blk = np.zeros(cfg.NBLK, np.int64)
        for gi, blks in enumerate(groups):
            gidx_of_blk[blks] = gi
        gof = gidx_of_blk[blk]
        for gi in range(len(groups)):
            for h in (0, 1):
                mm = (gof == gi) & (half == h)
                o = np.lexsort((es[mm], blk[mm]))
                eg[(c, gi, h)] = (es[mm][o], ed[mm][o], blk[mm][o])

    # ---- chunk counts per (group, half) = max over cores
    K = {}
    for gi in range(len(groups)):
        for h in (0, 1):
            n = max(len(eg[(c, gi, h)][0]) for c in range(NC))
            K[(gi, h)] = max(-(-n // P), 1)

    # ---- call table: per (group, half) split chunks into calls of <= 8
    calls = []          # (h, gi, chunk0, k)
    call_of_chunk = {}  # (gi, h, j) -> (ci, slot)
    for gi in range(len(groups)):
        for h in (0, 1):
            pos = 0
            while pos < K[(gi, h)]:
                k = min(8, K[(gi, h)] - pos)
                ci = len(calls)
                calls.append((h, gi, pos, k))
                for j in range(k):
                    call_of_chunk[(gi, h, pos + j)] = (ci, j)
                pos += k
    NCALLS = len(calls)

    # ---- matmul records: per (group, half, chunk) the envelope of blocks
    # touched by ANY core's real edges in that chunk. Record = (ci, slot, b).
    # Meta columns are assigned per call, contiguously.
    recs_of_call = [[] for _ in range(NCALLS)]   # list of (slot_in_call, b)
    for gi, blks in enumerate(groups):
        for h in (0, 1):
            for j in range(K[(gi, h)]):
                env = set()
                for c in range(NC):
                    blkarr = eg[(c, gi, h)][2]
                    seg = blkarr[j * P:(j + 1) * P]
                    env.update(seg.tolist())
                if not env:
                    env = {blks[-1]}
                ci, slot = call_of_chunk[(gi, h, j)]
                for b in sorted(env):
                    recs_of_call[ci].append((slot, b))
    # meta col offsets per call
    mc0 = np.zeros(NCALLS + 1, np.int64)
    for ci in range(NCALLS):
        assert len(recs_of_call[ci]) <= MAXREC, len(recs_of_call[ci])
        mc0[ci + 1] = mc0[ci] + len(recs_of_call[ci])
    MTOT = int(mc0[NCALLS])

    # per-block ordered record lists: (ci, slot_in_call, mslot)
    recs_of_block = {}
    for gi, blks in enumerate(groups):
        for b in blks:
            lst = []
            for h in (0, 1):
                for j in range(K[(gi, h)]):
                    ci, slot = call_of_chunk[(gi, h, j)]
                    for mslot, (rslot, rblk) in enumerate(recs_of_call[ci]):
                        if rslot == slot and rblk == b:
                            lst.append((ci, slot, mslot))
            recs_of_block[b] = lst

    # ---- per-core idx + meta tensors
    per_core_inputs = []
    for c in range(NC):
        idx = np.zeros((16, NCALLS * 64), np.int16)
        meta = np.full((P, MTOT), -999, np.int16)
        for ci, (h, gi, pos, k) in enumerate(calls):
            es, ed, blkarr = eg[(c, gi, h)]
            ni = k * P
            vals = np.zeros(ni, np.int64)
            lo, hi = pos * P, min((pos + k) * P, len(es))
            cnt = max(0, hi - lo)
            if cnt:
                vals[:cnt] = es[lo:hi] - (cfg.HALF if h else 0)
            ii = np.arange(ni)
            idx[ii % 16, ci * 64 + ii // 16] = vals.astype(np.int16)
            # meta: per record (slot, b): dstl - 128*b for that chunk's edges
            for mslot, (slot, b) in enumerate(recs_of_call[ci]):
                e0 = (pos + slot) * P
                e1 = min(e0 + P, len(es))
                if e1 > e0:
                    dv = ed[e0:e1] - b * P
                    col = np.full(P, -999, np.int64)
                    col[:e1 - e0] = np.where((dv >= 0) & (dv < P), dv, -999)
                    meta[:, mc0[ci] + mslot] = col.astype(np.int16)
        idxw = np.tile(idx, (8, 1))
        per_core_inputs.append({
            "gidx": idxw,
            "meta": meta,
            "dinvb": dinvb[c],
        })

    # pooling piece table per group
    pool_pieces = []
    seen = set()
    for gi, blks in enumerate(groups):
        n0g = blks[0] * P
        n1g = (blks[-1] + 1) * P
        pieces = []
        n = n0g
        while n < n1g:
            gl = n // cfg.GRAPH
            nend = min((gl + 1) * cfg.GRAPH, n1g)
            ft = gl not in seen
            seen.add(gl)
            pieces.append((n - n0g, nend - n0g, gl, ft))
            n = nend
        pool_pieces.append(pieces)

    # weights: zero-padded [128,128] lhsT variants
    wz = np.zeros((12, P, P), np.float32)
    for li, W in enumerate((W1, W2, W3)):
        W = np.asarray(W, np.float32)
        kdim = W.shape[0]
        nq = P // kdim
        for q in range(nq):
            half = q % 2
            wz[li * 4 + q, q * kdim:(q + 1) * kdim, half * COUT:(half + 1) * COUT] = W

    bias_col = np.zeros((P, 3), np.float32)
    for i, b in enumerate((b1, b2, b3)):
        bias_col[:, i] = np.tile(np.asarray(b, np.float32), P // COUT)

    # iota table [128, MAXREC*128] int16: col pattern 0..127 repeating
    iota = np.tile(np.arange(P, dtype=np.int16), MAXREC)[None, :].repeat(P, 0)

    ident = np.eye(P, dtype=np.float32)
    common = {
        "xp": Xp.astype(ml_dtypes.bfloat16),
        "wz": wz.astype(ml_dtypes.bfloat16),
        "biascol": bias_col,
        "id_f32": ident,
        "iotat": iota,
    }
    meta_info = dict(calls=calls, K=K, groups=groups, NCALLS=NCALLS,
                     recs_of_call=recs_of_call, recs_of_block=recs_of_block,
                     mc0=mc0, MTOT=MTOT, pool_pieces=pool_pieces)
    return common, per_core_inputs, meta_info


def build(cfg, meta):
    """Construct the Bass/Tile SPMD program."""
    calls, K, groups, NCALLS = meta["calls"], meta["K"], meta["groups"], meta["NCALLS"]
    recs_of_call, recs_of_block = meta["recs_of_call"], meta["recs_of_block"]
    mc0, MTOT, pool_pieces = meta["mc0"], meta["MTOT"], meta["pool_pieces"]
    NS, CH, CH1, T, COUT = cfg.NS, cfg.CH, cfg.CH1, cfg.T, cfg.COUT
    NS1 = max(CH1 // P, 1)
    CIN = cfg.CIN

    NQ = 4
    nc = bacc.Bacc("TRN2", target_bir_lowering=False, debug=False,
                   num_devices=cfg.NCORES, num_swdge_queues=NQ)

    xp = nc.dram_tensor("xp", [cfg.NPAD, CH1], BF16, kind="ExternalInput")
    gidx = nc.dram_tensor("gidx", [P, NCALLS * 64], I16, kind="ExternalInput")
    meta_d = nc.dram_tensor("meta", [P, MTOT], I16, kind="ExternalInput")
    dinvb_d = nc.dram_tensor("dinvb", [P, cfg.NBLK], F32, kind="ExternalInput")
    wz_d = nc.dram_tensor("wz", [12, P, P], BF16, kind="ExternalInput")
    biascol = nc.dram_tensor("biascol", [P, 3], F32, kind="ExternalInput")
    id_f32 = nc.dram_tensor("id_f32", [P, P], F32, kind="ExternalInput")
    iota_d = nc.dram_tensor("iotat", [P, MAXREC * P], I16, kind="ExternalInput")
    out = nc.dram_tensor("out", [P, 2 * NS * cfg.GPC], F32, kind="ExternalOutput")

    rg = [list(range(cfg.NCORES))]
    qstate = {"i": 0}

    with tile.TileContext(nc) as tc:
        with (
            tc.tile_pool(name="const", bufs=1) as constp,
            tc.tile_pool(name="msg", bufs=4) as msgp,
            tc.tile_pool(name="msgh", bufs=4) as msghp,
            tc.tile_pool(name="oh0", bufs=4) as ohp0,
            tc.tile_pool(name="oh1", bufs=4) as ohp1,
            tc.tile_pool(name="work", bufs=2) as workp,
            tc.tile_pool(name="psig", bufs=2) as psigp,
            tc.tile_pool(name="pool", bufs=1) as poolp,
            tc.tile_pool(name="gps", bufs=4, space="PSUM") as gpsp,
            tc.tile_pool(name="t1ps", bufs=1, space="PSUM") as t1psp,
            tc.tile_pool(name="psips", bufs=2, space="PSUM") as psipsp,
            tc.tile_pool(name="t2ps", bufs=1, space="PSUM") as t2psp,
            tc.tile_pool(name="dram", bufs=1, space="DRAM") as dramp,
        ):
            # ---- constants into SBUF
            idx_sb = constp.tile([P, NCALLS * 64], I16)
            nc.sync.dma_start(out=idx_sb[:], in_=gidx[:])
            meta_sb = constp.tile([P, MTOT], I16)
            nc.sync.dma_start(out=meta_sb[:], in_=meta_d[:])
            dinvb_sb = constp.tile([P, cfg.NBLK], F32)
            nc.sync.dma_start(out=dinvb_sb[:], in_=dinvb_d[:])
            iota_sb = constp.tile([P, MAXREC * P], I16)
            nc.sync.dma_start(out=iota_sb[:], in_=iota_d[:])
            wzt = constp.tile([P, 12 * P], BF16, tag="wzt")
            nc.sync.dma_start(
                out=wzt[:].rearrange("p (i m) -> p i m", i=12), in_=wz_d.ap().rearrange("i p m -> p i m"))
            bct = constp.tile([P, 3], F32)
            nc.sync.dma_start(out=bct[:], in_=biascol[:])
            idf = constp.tile([P, P], F32)
            nc.sync.dma_start(out=idf[:], in_=id_f32[:])

            # ---- pool accumulators
            lmax = poolp.tile([P, NS * cfg.GPC], F32, tag="lmax")
            lsum = poolp.tile([P, NS * cfg.GPC], F32, tag="lsum")
            fmax = poolp.tile([P, NS * cfg.GPC], F32, tag="fmax")
            fsum = poolp.tile([P, NS * cfg.GPC], F32, tag="fsum")
            for _t in (lmax, lsum, fmax, fsum):
                nc.vector.memset(_t[:], 0.0)

            # ---- DRAM intermediates
            h_mine = []
            h_full = []
            for i in range(2):
                hm = dramp.tile([cfg.NPC, CH], BF16, tag=f"hm{i}")
                h_mine.append(hm)
                hf = dramp.tile([cfg.NPAD, CH], BF16, tag=f"hf{i}")
                h_full.append(hf)

            calls_of_group = {}
            for ci, (h, gi, pos, k) in enumerate(calls):
                calls_of_group.setdefault(gi, []).append((ci, h, pos, k))

            def layer(li):
                ch_in = CH1 if li == 0 else CH
                ns_in = NS1 if li == 0 else NS
                if li == 0:
                    src_lo, src_hi = xp[:cfg.HALF, :], xp[cfg.HALF:cfg.NPAD, :]
                else:
                    hsrc = h_full[li - 1]
                    src_lo, src_hi = hsrc[:cfg.HALF, :], hsrc[cfg.HALF:cfg.NPAD, :]

                for gi, blks in enumerate(groups):
                    gtiles = {}
                    for ci, h, pos, k in calls_of_group[gi]:
                        ni = k * P
                        g = (msgp if h == 0 else msghp).tile([P, 8 * ch_in], BF16,
                                                             tag=f"m{h}")
                        q = qstate["i"] % NQ
                        qstate["i"] += 1
                        nc.gpsimd.dma_gather(
                            out_ap=g[:, :k * ch_in].rearrange("p (c e) -> p c e", e=ch_in),
                            in_ap=(src_lo if h == 0 else src_hi),
                            idxs_ap=idx_sb[:, ci * 64: ci * 64 + max(ni // 16, 1)],
                            num_idxs=ni,
                            num_idxs_reg=ni,
                            elem_size=ch_in,
                            queue_num=q,
                        )
                        # on-chip binary one-hot generation for this call
                        kmm = len(recs_of_call[ci])
                        oh = (ohp0 if h == 0 else ohp1).tile(
                            [P, MAXREC * P], BF16, tag=f"oh{h}")
                        nc.vector.tensor_tensor(
                            out=oh[:, :kmm * P].rearrange("p (m c) -> p m c", c=P),
                            in0=iota_sb[:, :kmm * P].rearrange("p (m c) -> p m c", c=P),
                            in1=meta_sb[:, int(mc0[ci]):int(mc0[ci]) + kmm]
                                .unsqueeze(2).broadcast_to([P, kmm, P]),
                            op=mybir.AluOpType.is_equal,
                        )
                        gtiles[ci] = (g, oh)

                    psi_grp = psigp.tile([P, NS * len(blks) * P], F32, tag="psig")
                    for bi, b in enumerate(blks):
                        gps = gpsp.tile([P, ch_in], F32, tag="gps")
                        recs = recs_of_block[b]
                        nmm = len(recs)
                        for mm, (ci, slot, mslot) in enumerate(recs):
                            g, oh = gtiles[ci]
                            nc.tensor.matmul(
                                gps[:],
                                lhsT=oh[:, mslot * P:(mslot + 1) * P],
                                rhs=g[:, slot * ch_in:(slot + 1) * ch_in],
                                start=(mm == 0), stop=(mm == nmm - 1),
                            )
                        # ---- epilogue: dinv[dst] scale folded into PSUM copy
                        gbf = workp.tile([P, ch_in], F32, tag="gbf")
                        nc.vector.tensor_scalar_mul(
                            gbf[:], gps[:], dinvb_sb[:, b:b + 1])
                        t1 = t1psp.tile([P, ns_in * P], F32, tag="t1")
                        for s in range(ns_in):
                            nc.tensor.transpose(
                                t1[:, s * P:(s + 1) * P],
                                gbf[:, s * P:(s + 1) * P], idf[:])
                        gt = workp.tile([P, ns_in * P], BF16, tag="gt")
                        nc.vector.tensor_copy(out=gt[:], in_=t1[:])
                        psi_ps = psipsp.tile([P, NS * P], F32, tag="psip")
                        kdim = CIN if li == 0 else COUT
                        nq = P // kdim
                        for t_ in range(T):
                            s_out = t_ // 2
                            q_ = t_ % nq
                            s_in = t_ // nq
                            nc.tensor.matmul(
                                psi_ps[:, s_out * P:(s_out + 1) * P],
                                lhsT=wzt[:, (li * 4 + q_) * P:(li * 4 + q_ + 1) * P],
                                rhs=gt[:, s_in * P:(s_in + 1) * P],
                                start=(t_ % 2 == 0), stop=(t_ % 2 == 1),
                            )
                        psi_sb = workp.tile([P, NS * P], F32, tag="psisb")
                        nc.scalar.activation(
                            psi_sb[:], psi_ps[:],
                            mybir.ActivationFunctionType.Relu,
                            bias=bct[:, li:li + 1],
                        )
                        gwk = len(blks) * P
                        dst_view = psi_grp[:].rearrange(
                            "p (s n) -> p s n", n=gwk)[:, :, bi * P:(bi + 1) * P]
                        nc.vector.tensor_copy(
                            out=dst_view,
                            in_=psi_sb[:].rearrange("p (s n) -> p s n", s=NS))
                        if li < 2:
                            t2 = t2psp.tile([P, NS * P], F32, tag="t2")
                            for s in range(NS):
                                nc.tensor.transpose(
                                    t2[:, s * P:(s + 1) * P],
                                    psi_grp[:, s * len(blks) * P + bi * P:
                                            s * len(blks) * P + (bi + 1) * P],
                                    idf[:])
                            hbf = workp.tile([P, CH], BF16, tag="hbf")
                            nc.vector.tensor_scalar_mul(
                                hbf[:], t2[:], dinvb_sb[:, b:b + 1])
                            nc.sync.dma_start(
                                out=h_mine[li][b * P:(b + 1) * P, :], in_=hbf[:])

                    # ---- pooling for this group
                    gw = len(blks) * P
                    for s in range(NS):
                        base = s * gw
                        for (n0, n1, gl, ft) in pool_pieces[gi]:
                            seg = psi_grp[:, base + n0: base + n1]
                            if ft:
                                nc.vector.reduce_max(
                                    out=lmax[:, s * cfg.GPC + gl: s * cfg.GPC + gl + 1],
                                    in_=seg, axis=mybir.AxisListType.X)
                                nc.vector.reduce_sum(
                                    out=lsum[:, s * cfg.GPC + gl: s * cfg.GPC + gl + 1],
                                    in_=seg, axis=mybir.AxisListType.X)
                            else:
                                tm = workp.tile([P, 2], F32, tag="ptmp")
                                nc.vector.reduce_max(out=tm[:, 0:1], in_=seg,
                                                     axis=mybir.AxisListType.X)
                                nc.vector.reduce_sum(out=tm[:, 1:2], in_=seg,
                                                     axis=mybir.AxisListType.X)
                                nc.vector.tensor_tensor(
                                    out=lmax[:, s * cfg.GPC + gl: s * cfg.GPC + gl + 1],
                                    in0=lmax[:, s * cfg.GPC + gl: s * cfg.GPC + gl + 1],
                                    in1=tm[:, 0:1], op=mybir.AluOpType.max)
                                nc.vector.tensor_add(
                                    out=lsum[:, s * cfg.GPC + gl: s * cfg.GPC + gl + 1],
                                    in0=lsum[:, s * cfg.GPC + gl: s * cfg.GPC + gl + 1],
                                    in1=tm[:, 1:2])

                    # early AllGather of first half once its blocks are written
                    if li < 2 and gi == ((cfg.NPC // 2) // P - 1) // cfg.GRP:
                        chk = cfg.NPC // 2
                        nc.gpsimd.collective_compute(
                            "AllGather", mybir.AluOpType.bypass,
                            replica_groups=rg,
                            ins=[h_mine[li][0:chk, :]],
                            outs=[h_full[li][0:cfg.HALF, :]],
                        )

                # ---- layer end: accumulate pools
                if li == 0:
                    nc.vector.tensor_copy(out=fmax[:], in_=lmax[:])
                    nc.vector.tensor_copy(out=fsum[:], in_=lsum[:])
                else:
                    nc.vector.tensor_add(out=fmax[:], in0=fmax[:], in1=lmax[:])
                    nc.vector.tensor_add(out=fsum[:], in0=fsum[:], in1=lsum[:])

                if li < 2:
                    chk = cfg.NPC // 2
                    nc.gpsimd.collective_compute(
                        "AllGather", mybir.AluOpType.bypass,
                        replica_groups=rg,
                        ins=[h_mine[li][chk:cfg.NPC, :]],
                        outs=[h_full[li][cfg.HALF:cfg.NPAD, :]],
                    )

            for _li in range(3):
                layer(_li)

            # mean = sum / GRAPH
            nc.vector.tensor_scalar_mul(fsum[:], fsum[:], float(np.float32(1.0 / cfg.GRAPH)))
            osb = workp.tile([P, 2 * NS * cfg.GPC], F32, tag="osb")
            nc.vector.tensor_copy(out=osb[:, :NS * cfg.GPC], in_=fmax[:])
            nc.vector.tensor_copy(out=osb[:, NS * cfg.GPC:], in_=fsum[:])
            nc.sync.dma_start(out=out[:], in_=osb[:])

    nc.compile()
    return nc


def unshard(cfg, results):
    """[NCORES][128, 2*NS*GPC] -> [B, 2*COUT, T] float32."""
    B, T, COUT, NS, GPC = cfg.B, cfg.T, cfg.COUT, cfg.NS, cfg.GPC
    out = np.zeros((B, 2 * COUT, T), np.float32)
    for c in range(cfg.NCORES):
        V = results[c]["out"]
        for gl in range(cfg.gpc[c]):
            g = cfg.goff[c] + gl
            for s in range(NS):
                for half in range(2):
                    t_ = 2 * s + half
                    co = np.arange(COUT)
                    pp = half * COUT + co
                    out[g, co, t_] = V[pp, s * GPC + gl]
                    out[g, COUT + co, t_] = V[pp, NS * GPC + s * GPC + gl]
    return out


_CACHE = {}


def kernel(**inputs):
    cfg = Cfg()
    common, per_core, meta = preprocess(
        cfg, inputs["x"], inputs["edge_index"], inputs["batch"],
        inputs["W1"], inputs["b1"], inputs["W2"], inputs["b2"],
        inputs["W3"], inputs["b3"])
    key = (meta["NCALLS"], meta["MTOT"])
    if key not in _CACHE:
        _CACHE[key] = build(cfg, meta)
    nc = _CACHE[key]
    in_maps = []
    for c in range(cfg.NCORES):
        m = dict(common)
        m.update(per_core[c])
        in_maps.append(m)
    res = run_bass_kernel_spmd(nc, in_maps, list(range(cfg.NCORES)))
    return unshard(cfg, res.results)


# revision 12
# speedup vs baseline: 1.8622x; 1.1401x over previous
"""Trainium2 Bass kernel for nn_GCNLayer (3-layer GCN + max/mean pooling, T temporal slices).

Self-contained: hardcodes the problem shapes (N=50000, E=800000, B=250, T=8,
CIN=32, COUT=64) and distributes over 8 NeuronCores by graph/dst-node range.

Algorithm per layer, with S = D^-1/2 (A+I) D^-1/2 and H' = D^-1/2 H:
    H_out = relu(dinv_dst * (sum_edges H'[src]) @ W + b),  H'_out = dinv * H_out
computed edge-parallel per core:
  - dma_gather of H'[src] rows (bf16, pre-scaled by dinv) on 4 SWDGE queues
  - scatter-add via BINARY one-hot matmul: lhsT one-hots are generated
    on-chip (DVE is_equal against an iota table), PSUM-accumulated per
    128-node dst block; per-block dinv[dst] scale folded into the PSUM copy
  - PE transpose -> W matmul (channels on partitions) -> relu+bias on ACT
  - pooling (max + mean over each graph's 200 nodes) via free-dim reduces
  - transpose back, scale by dinv, store bf16 H' to DRAM, AllGather halves
"""

import os
import numpy as np
import ml_dtypes

import concourse.bass as bass
import concourse.mybir as mybir
from concourse import bacc, tile
from concourse.bass_utils import run_bass_kernel_spmd

F32 = mybir.dt.float32
BF16 = mybir.dt.bfloat16
I16 = mybir.dt.int16
P = 128
MAXREC = 16          # max one-hot records per gather call


class Cfg:
    def __init__(self, N=50000, E=800000, B=250, T=8, CIN=32, COUT=64,
                 NCORES=8, GRAPH=200):
        self.N, self.E, self.B, self.T = N, E, B, T
        self.CIN, self.COUT, self.NCORES, self.GRAPH = CIN, COUT, NCORES, GRAPH
        base, rem = divmod(B, NCORES)
        self.gpc = [base + (1 if c < rem else 0) for c in range(NCORES)]
        self.GPC = max(self.gpc)                      # uniform per-core graph slots
        self.NPC = self.GPC * GRAPH                   # padded nodes per core
        assert self.NPC % P == 0
        self.NBLK = self.NPC // P                     # dst blocks per core
        self.NPAD = self.NPC * NCORES                 # padded global node count
        self.HALF = self.NPAD // 2                    # gather index split point
        assert self.HALF <= 32767 + 1
        self.CH1 = CIN * T                            # layer-1 feature row
        self.CH = COUT * T                            # layer-2/3 feature row
        assert self.CH % P == 0
        self.NS = self.CH // P                        # psi partition tiles (t-pairs)
        self.GRP = 4 if self.NBLK >= 4 else self.NBLK  # blocks per processing group
        self.goff = np.concatenate([[0], np.cumsum(self.gpc)]).astype(np.int64)
        self.nstart = [c * self.NPC for c in range(NCORES)]


def preprocess(cfg, x, edge_index, batch, W1, b1, W2, b2, W3, b3):
    """Build all per-core device inputs. Returns (common_inputs, per_core_inputs, meta)."""
    N, E, T, CIN, COUT = cfg.N, cfg.E, cfg.T, cfg.CIN, cfg.COUT
    NC = cfg.NCORES
    src = np.asarray(edge_index[0], np.int64)
    dst = np.asarray(edge_index[1], np.int64)

    deg = np.bincount(dst, minlength=N).astype(np.float32) + 1.0
    dinv = (1.0 / np.sqrt(deg)).astype(np.float32)

    batch = np.asarray(batch, np.int64)
    g2c = np.zeros(cfg.B, np.int64)
    for c in range(NC):
        g2c[cfg.goff[c]:cfg.goff[c + 1]] = c
    node_core = g2c[batch]
    first_node_of_core = np.array([cfg.goff[c] * cfg.GRAPH for c in range(NC)], np.int64)
    local_n = np.arange(N) - first_node_of_core[node_core]
    CHK = cfg.NPC // 2
    pad_id = np.where(local_n < CHK,
                      node_core * CHK + local_n,
                      cfg.HALF + node_core * CHK + (local_n - CHK))

    srcp = pad_id[src]
    dstc = node_core[dst]
    dstl = local_n[dst]

    # X permuted to [NPAD, T*CIN] rows, PRE-SCALED by dinv (H' convention)
    Xp = np.zeros((cfg.NPAD, cfg.CH1), np.float32)
    xm = np.moveaxis(np.asarray(x, np.float32), 2, 1).reshape(N, T * CIN)
    Xp[pad_id] = xm * dinv[:, None]

    # dinv per core block layout [128, NBLK]: dinv of node b*128+p (1.0 for pads)
    dinvb = np.ones((NC, P, cfg.NBLK), np.float32)
    for c in range(NC):
        ln = np.where(node_core == c)[0]
        dv = np.ones(cfg.NPC, np.float32)
        dv[local_n[ln]] = dinv[ln]
        dinvb[c] = dv.reshape(cfg.NBLK, P).T

    # groups of blocks
    groups = []
    b0 = 0
    while b0 < cfg.NBLK:
        groups.append(list(range(b0, min(b0 + cfg.GRP, cfg.NBLK))))
        b0 += cfg.GRP

    # ---- per-(core, group, half) sorted edge lists (src pad-id, dst local)
    # Self-loops included as ordinary binary edges.
    eg = {}
    for c in range(NC):
        m = dstc == c
        es, ed = srcp[m], dstl[m]
        ln = np.where(node_core == c)[0]
        es = np.concatenate([es, pad_id[ln]])
        ed = np.concatenate([ed, local_n[ln]])
        blk = ed // P
        half = (es >= cfg.HALF).astype(np.int64)
        gidx_of_blk = np.zeros(cfg.NBLK, np.int64)
        for gi, blks in enumerate(groups):
            gidx_of_blk[blks] = gi
        gof = gidx_of_blk[blk]
        for gi in range(len(groups)):
            for h in (0, 1):
                mm = (gof == gi) & (half == h)
                o = np.lexsort((es[mm], blk[mm]))
                eg[(c, gi, h)] = (es[mm][o], ed[mm][o], blk[mm][o])

    # ---- chunk counts per (group, half) = max over cores
    K = {}
    for gi in range(len(groups)):
        for h in (0, 1):
            n = max(len(eg[(c, gi, h)][0]) for c in range(NC))
            K[(gi, h)] = max(-(-n // P), 1)

    # ---- call table: per (group, half) split chunks into calls of <= 8
    calls = []          # (h, gi, chunk0, k)
    call_of_chunk = {}  # (gi, h, j) -> (ci, slot)
    for gi in range(len(groups)):
        for h in (0, 1):
            pos = 0
            while pos < K[(gi, h)]:
                k = min(8, K[(gi, h)] - pos)
                ci = len(calls)
                calls.append((h, gi, pos, k))
                for j in range(k):
                    call_of_chunk[(gi, h, pos + j)] = (ci, j)
                pos += k
    NCALLS = len(calls)

    # ---- matmul records: per (group, half, chunk) the envelope of blocks
    # touched by ANY core's real edges in that chunk. Record = (ci, slot, b).
    # Meta columns are assigned per call, contiguously.
    recs_of_call = [[] for _ in range(NCALLS)]   # list of (slot_in_call, b)
    for gi, blks in enumerate(groups):
        for h in (0, 1):
            for j in range(K[(gi, h)]):
                env = set()
                for c in range(NC):
                    blkarr = eg[(c, gi, h)][2]
                    seg = blkarr[j * P:(j + 1) * P]
                    env.update(seg.tolist())
                if not env:
                    env = {blks[-1]}
                ci, slot = call_of_chunk[(gi, h, j)]
                for b in sorted(env):
                    recs_of_call[ci].append((slot, b))
    # meta col offsets per call
    mc0 = np.zeros(NCALLS + 1, np.int64)
    for ci in range(NCALLS):
        assert len(recs_of_call[ci]) <= MAXREC, len(recs_of_call[ci])
        mc0[ci + 1] = mc0[ci] + len(recs_of_call[ci])
    MTOT = int(mc0[NCALLS])

    # per-block ordered record lists: (ci, slot_in_call, mslot)
    recs_of_block = {}
    for gi, blks in enumerate(groups):
        for b in blks:
            lst = []
            for h in (0, 1):
                for j in range(K[(gi, h)]):
                    ci, slot = call_of_chunk[(gi, h, j)]
                    for mslot, (rslot, rblk) in enumerate(recs_of_call[ci]):
                        if rslot == slot and rblk == b:
                            lst.append((ci, slot, mslot))
            recs_of_block[b] = lst

    # ---- per-core idx + meta tensors
    per_core_inputs = []
    for c in range(NC):
        idx = np.zeros((16, NCALLS * 64), np.int16)
        meta = np.full((P, MTOT), -999, np.float32)
        for ci, (h, gi, pos, k) in enumerate(calls):
            es, ed, blkarr = eg[(c, gi, h)]
            ni = k * P
            vals = np.zeros(ni, np.int64)
            lo, hi = pos * P, min((pos + k) * P, len(es))
            cnt = max(0, hi - lo)
            if cnt:
                vals[:cnt] = es[lo:hi] - (cfg.HALF if h else 0)
            ii = np.arange(ni)
            idx[ii % 16, ci * 64 + ii // 16] = vals.astype(np.int16)
            # meta: per record (slot, b): dstl - 128*b for that chunk's edges
            for mslot, (slot, b) in enumerate(recs_of_call[ci]):
                e0 = (pos + slot) * P
                e1 = min(e0 + P, len(es))
                if e1 > e0:
                    dv = ed[e0:e1] - b * P
                    col = np.full(P, -999, np.int64)
                    col[:e1 - e0] = np.where((dv >= 0) & (dv < P), dv, -999)
                    meta[:, mc0[ci] + mslot] = col.astype(np.float32)
        idxw = np.tile(idx, (8, 1))
        per_core_inputs.append({
            "gidx": idxw,
            "meta": meta.astype(ml_dtypes.bfloat16),
            "dinvb": dinvb[c],
        })

    # pooling piece table per group
    pool_pieces = []
    seen = set()
    for gi, blks in enumerate(groups):
        n0g = blks[0] * P
        n1g = (blks[-1] + 1) * P
        pieces = []
        n = n0g
        while n < n1g:
            gl = n // cfg.GRAPH
            nend = min((gl + 1) * cfg.GRAPH, n1g)
            ft = gl not in seen
            seen.add(gl)
            pieces.append((n - n0g, nend - n0g, gl, ft))
            n = nend
        pool_pieces.append(pieces)

    # weights: zero-padded [128,128] lhsT variants
    wz = np.zeros((12, P, P), np.float32)
    for li, W in enumerate((W1, W2, W3)):
        W = np.asarray(W, np.float32)
        kdim = W.shape[0]
        nq = P // kdim
        for q in range(nq):
            half = q % 2
            wz[li * 4 + q, q * kdim:(q + 1) * kdim, half * COUT:(half + 1) * COUT] = W

    bias_col = np.zeros((P, 3), np.float32)
    for i, b in enumerate((b1, b2, b3)):
        bias_col[:, i] = np.tile(np.asarray(b, np.float32), P // COUT)

    # iota table [128, MAXREC*128] int16: col pattern 0..127 repeating
    iota = np.tile(np.arange(P, dtype=np.float32), MAXREC)[None, :].repeat(P, 0).astype(ml_dtypes.bfloat16)

    ident = np.eye(P, dtype=np.float32)
    common = {
        "xp": Xp.astype(ml_dtypes.bfloat16),
        "wz": wz.astype(ml_dtypes.bfloat16),
        "biascol": bias_col,
        "id_f32": ident,
        "iotat": iota,
    }
    meta_info = dict(calls=calls, K=K, groups=groups, NCALLS=NCALLS,
                     recs_of_call=recs_of_call, recs_of_block=recs_of_block,
                     mc0=mc0, MTOT=MTOT, pool_pieces=pool_pieces)
    return common, per_core_inputs, meta_info


def build(cfg, meta):
    """Construct the Bass/Tile SPMD program."""
    calls, K, groups, NCALLS = meta["calls"], meta["K"], meta["groups"], meta["NCALLS"]
    recs_of_call, recs_of_block = meta["recs_of_call"], meta["recs_of_block"]
    mc0, MTOT, pool_pieces = meta["mc0"], meta["MTOT"], meta["pool_pieces"]
    NS, CH, CH1, T, COUT = cfg.NS, cfg.CH, cfg.CH1, cfg.T, cfg.COUT
    NS1 = max(CH1 // P, 1)
    CIN = cfg.CIN

    NQ = 4
    nc = bacc.Bacc("TRN2", target_bir_lowering=False, debug=False,
                   num_devices=cfg.NCORES, num_swdge_queues=NQ)

    xp = nc.dram_tensor("xp", [cfg.NPAD, CH1], BF16, kind="ExternalInput")
    gidx = nc.dram_tensor("gidx", [P, NCALLS * 64], I16, kind="ExternalInput")
    meta_d = nc.dram_tensor("meta", [P, MTOT], BF16, kind="ExternalInput")
    dinvb_d = nc.dram_tensor("dinvb", [P, cfg.NBLK], F32, kind="ExternalInput")
    wz_d = nc.dram_tensor("wz", [12, P, P], BF16, kind="ExternalInput")
    biascol = nc.dram_tensor("biascol", [P, 3], F32, kind="ExternalInput")
    id_f32 = nc.dram_tensor("id_f32", [P, P], F32, kind="ExternalInput")
    iota_d = nc.dram_tensor("iotat", [P, MAXREC * P], BF16, kind="ExternalInput")
    out = nc.dram_tensor("out", [P, 2 * NS * cfg.GPC], F32, kind="ExternalOutput")

    rg = [list(range(cfg.NCORES))]
    qstate = {"i": 0}

    with tile.TileContext(nc) as tc:
        with (
            tc.tile_pool(name="const", bufs=1) as constp,
            tc.tile_pool(name="msg", bufs=5) as msgp,
            tc.tile_pool(name="msgh", bufs=5) as msghp,
            tc.tile_pool(name="oh0", bufs=6) as ohp0,
            tc.tile_pool(name="oh1", bufs=6) as ohp1,
            tc.tile_pool(name="work", bufs=2) as workp,
            tc.tile_pool(name="psig", bufs=2) as psigp,
            tc.tile_pool(name="pool", bufs=1) as poolp,
            tc.tile_pool(name="gps", bufs=4, space="PSUM") as gpsp,
            tc.tile_pool(name="t1ps", bufs=1, space="PSUM") as t1psp,
            tc.tile_pool(name="psips", bufs=2, space="PSUM") as psipsp,
            tc.tile_pool(name="t2ps", bufs=1, space="PSUM") as t2psp,
            tc.tile_pool(name="dram", bufs=1, space="DRAM") as dramp,
        ):
            # ---- constants into SBUF
            idx_sb = constp.tile([P, NCALLS * 64], I16)
            nc.sync.dma_start(out=idx_sb[:], in_=gidx[:])
            meta_sb = constp.tile([P, MTOT], BF16)
            nc.sync.dma_start(out=meta_sb[:], in_=meta_d[:])
            dinvb_sb = constp.tile([P, cfg.NBLK], F32)
            nc.sync.dma_start(out=dinvb_sb[:], in_=dinvb_d[:])
            iota_sb = constp.tile([P, MAXREC * P], BF16)
            nc.sync.dma_start(out=iota_sb[:], in_=iota_d[:])
            wzt = constp.tile([P, 12 * P], BF16, tag="wzt")
            nc.sync.dma_start(
                out=wzt[:].rearrange("p (i m) -> p i m", i=12), in_=wz_d.ap().rearrange("i p m -> p i m"))
            bct = constp.tile([P, 3], F32)
            nc.sync.dma_start(out=bct[:], in_=biascol[:])
            idf = constp.tile([P, P], F32)
            nc.sync.dma_start(out=idf[:], in_=id_f32[:])

            # ---- pool accumulators
            lmax = poolp.tile([P, NS * cfg.GPC], F32, tag="lmax")
            lsum = poolp.tile([P, NS * cfg.GPC], F32, tag="lsum")
            fmax = poolp.tile([P, NS * cfg.GPC], F32, tag="fmax")
            fsum = poolp.tile([P, NS * cfg.GPC], F32, tag="fsum")
            for _t in (lmax, lsum, fmax, fsum):
                nc.vector.memset(_t[:], 0.0)

            # ---- DRAM intermediates
            h_mine = []
            h_full = []
            for i in range(2):
                hm = dramp.tile([cfg.NPC, CH], BF16, tag=f"hm{i}")
                h_mine.append(hm)
                hf = dramp.tile([cfg.NPAD, CH], BF16, tag=f"hf{i}")
                h_full.append(hf)

            calls_of_group = {}
            for ci, (h, gi, pos, k) in enumerate(calls):
                calls_of_group.setdefault(gi, []).append((ci, h, pos, k))

            def layer(li):
                ch_in = CH1 if li == 0 else CH
                ns_in = NS1 if li == 0 else NS
                if li == 0:
                    src_lo, src_hi = xp[:cfg.HALF, :], xp[cfg.HALF:cfg.NPAD, :]
                else:
                    hsrc = h_full[li - 1]
                    src_lo, src_hi = hsrc[:cfg.HALF, :], hsrc[cfg.HALF:cfg.NPAD, :]

                for gi, blks in enumerate(groups):
                    gtiles = {}
                    for ci, h, pos, k in calls_of_group[gi]:
                        ni = k * P
                        g = (msgp if h == 0 else msghp).tile([P, 8 * ch_in], BF16,
                                                             tag=f"m{h}")
                        q = qstate["i"] % NQ
                        qstate["i"] += 1
                        nc.gpsimd.dma_gather(
                            out_ap=g[:, :k * ch_in].rearrange("p (c e) -> p c e", e=ch_in),
                            in_ap=(src_lo if h == 0 else src_hi),
                            idxs_ap=idx_sb[:, ci * 64: ci * 64 + max(ni // 16, 1)],
                            num_idxs=ni,
                            num_idxs_reg=ni,
                            elem_size=ch_in,
                            queue_num=q,
                        )
                        # on-chip binary one-hot generation for this call
                        kmm = len(recs_of_call[ci])
                        oh = (ohp0 if h == 0 else ohp1).tile(
                            [P, MAXREC * P], BF16, tag=f"oh{h}")
                        nc.vector.tensor_tensor(
                            out=oh[:, :kmm * P].rearrange("p (m c) -> p m c", c=P),
                            in0=iota_sb[:, :kmm * P].rearrange("p (m c) -> p m c", c=P),
                            in1=meta_sb[:, int(mc0[ci]):int(mc0[ci]) + kmm]
                                .unsqueeze(2).broadcast_to([P, kmm, P]),
                            op=mybir.AluOpType.is_equal,
                        )
                        gtiles[ci] = (g, oh)

                    psi_grp = psigp.tile([P, NS * len(blks) * P], F32, tag="psig")
                    for bi, b in enumerate(blks):
                        gps = gpsp.tile([P, ch_in], F32, tag="gps")
                        recs = recs_of_block[b]
                        nmm = len(recs)
                        for mm, (ci, slot, mslot) in enumerate(recs):
                            g, oh = gtiles[ci]
                            nc.tensor.matmul(
                                gps[:],
                                lhsT=oh[:, mslot * P:(mslot + 1) * P],
                                rhs=g[:, slot * ch_in:(slot + 1) * ch_in],
                                start=(mm == 0), stop=(mm == nmm - 1),
                            )
                        # ---- epilogue: dinv[dst] scale folded into PSUM copy
                        gbf = workp.tile([P, ch_in], F32, tag="gbf")
                        nc.vector.tensor_scalar_mul(
                            gbf[:], gps[:], dinvb_sb[:, b:b + 1])
                        t1 = t1psp.tile([P, ns_in * P], F32, tag="t1")
                        for s in range(ns_in):
                            nc.tensor.transpose(
                                t1[:, s * P:(s + 1) * P],
                                gbf[:, s * P:(s + 1) * P], idf[:])
                        gt = workp.tile([P, ns_in * P], BF16, tag="gt")
                        nc.vector.tensor_copy(out=gt[:], in_=t1[:])
                        psi_ps = psipsp.tile([P, NS * P], F32, tag="psip")
                        kdim = CIN if li == 0 else COUT
                        nq = P // kdim
                        for t_ in range(T):
                            s_out = t_ // 2
                            q_ = t_ % nq
                            s_in = t_ // nq
                            nc.tensor.matmul(
                                psi_ps[:, s_out * P:(s_out + 1) * P],
                                lhsT=wzt[:, (li * 4 + q_) * P:(li * 4 + q_ + 1) * P],
                                rhs=gt[:, s_in * P:(s_in + 1) * P],
                                start=(t_ % 2 == 0), stop=(t_ % 2 == 1),
                            )
                        gwk = len(blks) * P
                        dst_view = psi_grp[:].rearrange(
                            "p (s n) -> p s n", n=gwk)[:, :, bi * P:(bi + 1) * P]
                        nc.scalar.activation(
                            dst_view,
                            psi_ps[:].rearrange("p (s n) -> p s n", s=NS),
                            mybir.ActivationFunctionType.Relu,
                            bias=bct[:, li:li + 1],
                        )
                        if li < 2:
                            t2 = t2psp.tile([P, NS * P], F32, tag="t2")
                            for s in range(NS):
                                nc.tensor.transpose(
                                    t2[:, s * P:(s + 1) * P],
                                    psi_grp[:, s * len(blks) * P + bi * P:
                                            s * len(blks) * P + (bi + 1) * P],
                                    idf[:])
                            hbf = workp.tile([P, CH], BF16, tag="hbf")
                            nc.vector.tensor_scalar_mul(
                                hbf[:], t2[:], dinvb_sb[:, b:b + 1])
                            nc.sync.dma_start(
                                out=h_mine[li][b * P:(b + 1) * P, :], in_=hbf[:])

                    # ---- pooling for this group
                    gw = len(blks) * P
                    for s in range(NS):
                        base = s * gw
                        for (n0, n1, gl, ft) in pool_pieces[gi]:
                            seg = psi_grp[:, base + n0: base + n1]
                            if ft:
                                nc.vector.reduce_max(
                                    out=lmax[:, s * cfg.GPC + gl: s * cfg.GPC + gl + 1],
                                    in_=seg, axis=mybir.AxisListType.X)
                                nc.vector.reduce_sum(
                                    out=lsum[:, s * cfg.GPC + gl: s * cfg.GPC + gl + 1],
                                    in_=seg, axis=mybir.AxisListType.X)
                            else:
                                tm = workp.tile([P, 2], F32, tag="ptmp")
                                nc.vector.reduce_max(out=tm[:, 0:1], in_=seg,
                                                     axis=mybir.AxisListType.X)
                                nc.vector.reduce_sum(out=tm[:, 1:2], in_=seg,
                                                     axis=mybir.AxisListType.X)
                                nc.vector.tensor_tensor(
                                    out=lmax[:, s * cfg.GPC + gl: s * cfg.GPC + gl + 1],
                                    in0=lmax[:, s * cfg.GPC + gl: s * cfg.GPC + gl + 1],
                                    in1=tm[:, 0:1], op=mybir.AluOpType.max)
                                nc.vector.tensor_add(
                                    out=lsum[:, s * cfg.GPC + gl: s * cfg.GPC + gl + 1],
                                    in0=lsum[:, s * cfg.GPC + gl: s * cfg.GPC + gl + 1],
                                    in1=tm[:, 1:2])

                    # early AllGather of first half once its blocks are written
                    if li < 2 and gi == ((cfg.NPC // 2) // P - 1) // cfg.GRP:
                        chk = cfg.NPC // 2
                        nc.gpsimd.collective_compute(
                            "AllGather", mybir.AluOpType.bypass,
                            replica_groups=rg,
                            ins=[h_mine[li][0:chk, :]],
                            outs=[h_full[li][0:cfg.HALF, :]],
                        )

                # ---- layer end: accumulate pools
                if li == 0:
                    nc.vector.tensor_copy(out=fmax[:], in_=lmax[:])
                    nc.vector.tensor_copy(out=fsum[:], in_=lsum[:])
                else:
                    nc.vector.tensor_add(out=fmax[:], in0=fmax[:], in1=lmax[:])
                    nc.vector.tensor_add(out=fsum[:], in0=fsum[:], in1=lsum[:])

                if li < 2:
                    chk = cfg.NPC // 2
                    nc.gpsimd.collective_compute(
                        "AllGather", mybir.AluOpType.bypass,
                        replica_groups=rg,
                        ins=[h_mine[li][chk:cfg.NPC, :]],
                        outs=[h_full[li][cfg.HALF:cfg.NPAD, :]],
                    )

            for _li in range(3):
                layer(_li)

            # mean = sum / GRAPH
            nc.vector.tensor_scalar_mul(fsum[:], fsum[:], float(np.float32(1.0 / cfg.GRAPH)))
            osb = workp.tile([P, 2 * NS * cfg.GPC], F32, tag="osb")
            nc.vector.tensor_copy(out=osb[:, :NS * cfg.GPC], in_=fmax[:])
            nc.vector.tensor_copy(out=osb[:, NS * cfg.GPC:], in_=fsum[:])
            nc.sync.dma_start(out=out[:], in_=osb[:])

    nc.compile()
    return nc


def unshard(cfg, results):
    """[NCORES][128, 2*NS*GPC] -> [B, 2*COUT, T] float32."""
    B, T, COUT, NS, GPC = cfg.B, cfg.T, cfg.COUT, cfg.NS, cfg.GPC
    out = np.zeros((B, 2 * COUT, T), np.float32)
    for c in range(cfg.NCORES):
        V = results[c]["out"]
        for gl in range(cfg.gpc[c]):
            g = cfg.goff[c] + gl
            for s in range(NS):
                for half in range(2):
                    t_ = 2 * s + half
                    co = np.arange(COUT)
                    pp = half * COUT + co
                    out[g, co, t_] = V[pp, s * GPC + gl]
                    out[g, COUT + co, t_] = V[pp, NS * GPC + s * GPC + gl]
    return out


_CACHE = {}


def kernel(**inputs):
    cfg = Cfg()
    common, per_core, meta = preprocess(
        cfg, inputs["x"], inputs["edge_index"], inputs["batch"],
        inputs["W1"], inputs["b1"], inputs["W2"], inputs["b2"],
        inputs["W3"], inputs["b3"])
    key = (meta["NCALLS"], meta["MTOT"])
    if key not in _CACHE:
        _CACHE[key] = build(cfg, meta)
    nc = _CACHE[key]
    in_maps = []
    for c in range(cfg.NCORES):
        m = dict(common)
        m.update(per_core[c])
        in_maps.append(m)
    res = run_bass_kernel_spmd(nc, in_maps, list(range(cfg.NCORES)))
    return unshard(cfg, res.results)
